# revision 1
# baseline (speedup 1.0000x reference)
"""CodonAttention Trainium2 kernel (fp16 stream, issue-lean pipeline).

Math (per batch b, head h):
  q = x @ wq.T + bq ; k = x @ wk.T + bk ; v = x @ wv.T + bv   (head slices)
  scores = q k^T / 8 + syn_bias[codons_i, codons_j]
  out    = softmax(scores) @ v ;  final = concat_heads(out) @ wo.T + bo

Key algebraic trick: the pairwise codon bias factors through one-hots,
  pair_bias = onehot @ syn_bias @ onehot.T
so augmenting q' = [(q+bq)/8 | onehot @ syn_bias] and k' = [k | onehot] gives
  scores = q' @ k'.T        (effective head dim 128 — exactly one partition)
Softmax runs without max-subtraction (|scores| <= ~4.3, exp safe in fp32) and
the row-sum l is obtained with a ones-column in v: [O | l] = P @ [v | 1].

Sharding: 8 cores = (batch b in {0,1}) x (head h in {0..3}). Each core runs
the full attention for its (b, h), producing the UNNORMALIZED partial
projection outT = (wo_h @ O_h.T) (256, 4096) plus denominators lT (1, 4096);
the host divides, sums the 4 head partials per batch, transposes, adds bo.

Profile-driven design (trace facts from this hardware):
- Phase B is ACT-bound: exp runs 1 col/cycle @1.2GHz regardless of dtype,
  ~1.0us per [128,1024] group; the PE streams 512-row fp16 matmuls at
  ~0.42 ns/row so 4 matmuls/group (~0.87us) fit under the exp.
- Every dma_start costs ~0.6-1us of *issue* time on its queue, so DMA
  issues are spread: Sync + Scalar queues carry x/weights (hwdge),
  GpSimd carries the bias streams and all output DMAs (swdge, idle
  engine). Queue order puts chunk 0 first so compute starts ~3us in.
- Engines execute their queue in order, so late-chunk work must not sit
  in front of the attention stream: q/k projections and v transposes
  for chunks 3..7 are injected INTO the qb-0 attention stream right
  before the groups that consume them.
- The per-block output projection is deferred two groups into the next
  query block so its oacc->oall->PE chain never stalls the score
  pipeline (it runs in loose slots, PSUM bank shared with the qk
  projection pool).
- PSUM budget (8 banks): scores double-buffer 2x2 + oacc 2 + v-flip 1 +
  qkproj/outproj shared 1.
"""

import numpy as np

import concourse.mybir as mybir
import concourse.tile as tile
from concourse import bacc
from concourse.bass_utils import run_bass_kernel_spmd


def _ensure_axon_ntff_hook():
    """This image's antenv package lacks axon_hooks, so
    run_bass_kernel_spmd(trace=True) (or BASS_TRACE=1) would die on the
    import. Register a compatible module backed by the libaxon_pjrt C ABI
    so tracing works if a caller requests it."""
    import sys
    try:
        import antenv.axon_hooks  # noqa: F401
        return
    except ImportError:
        pass
    import contextlib
    import ctypes
    import types
    try:
        lib = ctypes.CDLL("/opt/axon/libaxon_pjrt.so")
        has = hasattr(lib, "axon_start_nrt_profile")
    except OSError:
        has = False
    if has:
        lib.axon_start_nrt_profile.argtypes = [ctypes.POINTER(ctypes.c_int64),
                                               ctypes.c_size_t]
        lib.axon_start_nrt_profile.restype = ctypes.c_int64
        lib.axon_stop_nrt_profile.argtypes = [ctypes.c_char_p]
        lib.axon_stop_nrt_profile.restype = ctypes.c_int64

        @contextlib.contextmanager
        def _hook(output_dir, device_ids):
            import jax
            jax.devices()
            if device_ids:
                ids = (ctypes.c_int64 * len(device_ids))(*device_ids)
                rc = lib.axon_start_nrt_profile(ids, len(device_ids))
            else:
                rc = lib.axon_start_nrt_profile(None, 0)
            if rc != 0:
                raise RuntimeError(f"axon_start_nrt_profile rc={rc}")
            try:
                yield
            finally:
                lib.axon_stop_nrt_profile(str(output_dir).encode())
    else:
        _hook = None

    mod = types.ModuleType("antenv.axon_hooks")
    _state = {"hook": _hook}
    mod.get_axon_ntff_profile_hook = lambda: _state["hook"]
    mod.set_axon_ntff_profile_hook = lambda h: _state.__setitem__("hook", h)
    sys.modules["antenv.axon_hooks"] = mod


_ensure_axon_ntff_hook()

B, S, HID, NH, D = 2, 4096, 256, 4, 64
DV = D + 4         # v cols + ones column + 3 pad
LCOL = D           # index of the ones column inside a v tile
QB = 512           # query block (free dim of score matmuls)
KT = 128           # key tile (partition dim of transposed scores)
CH = 512           # x chunk width
NCH = S // CH      # 8
NQB = S // QB      # 8
NKT = S // KT      # 32
GRP = 2            # key tiles per exp group (2 PSUM banks per group)
NG = NKT // GRP    # 16 groups per query block

F32 = mybir.dt.float32
F32R = mybir.dt.float32r
F16 = mybir.dt.float16
Exp = mybir.ActivationFunctionType.Exp


def build_program():
    nc = bacc.Bacc("TRN2", target_bir_lowering=False, debug=False, num_devices=8)

    def di(name, shape, dt=F16):
        return nc.dram_tensor(name, shape, dt, kind="ExternalInput").ap()

    xT = di("xT", [HID, S])            # x[b].T
    wqk = di("wqk", [128, 256])        # [wqT_hi/8|wkT_hi ; wqT_lo/8|wkT_lo]
    wv2 = di("wv2", [128, 2 * DV])     # [wvT_hi | wvT_lo], col 64.. pad 0
    bias2 = di("bias2", [128, 2], F32) # col0 = [bq/8; bk], col1 = [bv;1;0..]
    bsynT = di("bsynT", [D, S])        # (onehot @ syn_bias).T
    onehotT = di("onehotT", [D, S])
    woT = di("woT", [D, HID])          # wo[:, hslice].T
    idm = di("idm", [128, 128], F32R)  # identity for TensorE transpose
    outT = nc.dram_tensor("outT", [HID, S], F16, kind="ExternalOutput").ap()
    lT = nc.dram_tensor("lT", [1, S], F32, kind="ExternalOutput").ap()

    with tile.TileContext(nc) as tc:
        _body(tc, xT, wqk, wv2, bias2, bsynT, onehotT, woT, idm, outT, lT)
    nc.compile()
    return nc


def _body(tc, xT, wqk, wv2, bias2, bsynT, onehotT, woT, idm, outT, lT):
    nc = tc.nc
    mm = nc.tensor.matmul

    with (
        tc.tile_pool(name="const", bufs=1) as constp,
        tc.tile_pool(name="big", bufs=1) as bigp,
        tc.tile_pool(name="vts", bufs=2) as vtsp,
        tc.tile_pool(name="pt", bufs=9) as ptp,
        tc.tile_pool(name="ob", bufs=2) as obp,
    ):
        # ---- constants ----
        wqk_sb = constp.tile([128, 256], F16, name="wqk_sb", tag="wqk_sb")
        wv_sb = constp.tile([128, 2 * DV], F16, name="wv_sb", tag="wv_sb")
        b2_sb = constp.tile([128, 2], F32, name="b2_sb", tag="b2_sb")
        wo_sb = constp.tile([D, HID], F16, name="wo_sb", tag="wo_sb")
        id_sb = constp.tile([128, 128], F32R, name="id_sb", tag="id_sb")
        scr = constp.tile([1, 1], F32, name="scr", tag="scr")

        # persistent activations (subregion deps make slices per-chunk)
        xc0 = [bigp.tile([128, CH], F16, name=f"xc0_{c}", tag=f"xc0_{c}")
               for c in range(NCH)]
        xc1 = [bigp.tile([128, CH], F16, name=f"xc1_{c}", tag=f"xc1_{c}")
               for c in range(NCH)]
        qTt = bigp.tile([128, S], F16, name="qTt", tag="qTt")  # 0:64 q/8, 64:128 bsynT
        kTt = bigp.tile([128, S], F16, name="kTt", tag="kTt")  # 0:64 k,   64:128 onehotT
        vb = bigp.tile([128, NKT * DV], F16, name="vb", tag="vb")  # v' key-major
        oall = bigp.tile([D, S], F16, name="oall", tag="oall")
        l_sb = bigp.tile([1, S], F32, name="l_sb", tag="l_sb")

        # ---- DMA issues, spread across queues, chunk 0 first ----
        nc.sync.dma_start(wqk_sb[:], wqk[:])
        nc.sync.dma_start(b2_sb[:], bias2[:])
        for c in (0, 2, 4, 6):
            cs = slice(c * CH, (c + 1) * CH)
            nc.sync.dma_start(xc0[c][:], xT[0:128, cs])
            nc.sync.dma_start(xc1[c][:], xT[128:256, cs])
        nc.sync.dma_start(id_sb[:], idm[:])
        nc.scalar.dma_start(wv_sb[:], wv2[:])
        nc.scalar.dma_start(wo_sb[:], woT[:])
        for c in (1, 3, 5, 7):
            cs = slice(c * CH, (c + 1) * CH)
            nc.scalar.dma_start(xc0[c][:], xT[0:128, cs])
            nc.scalar.dma_start(xc1[c][:], xT[128:256, cs])
        for c in range(4):
            cs = slice(c * CH, (c + 1) * CH)
            nc.gpsimd.dma_start(qTt[64:128, cs], bsynT[:, cs])
            nc.gpsimd.dma_start(kTt[64:128, cs], onehotT[:, cs])

        # warm the ACT exp table (~2.7us) while projections run
        nc.scalar.activation(scr[:], b2_sb[0:1, 0:1], Exp)

        # late bias chunks ride behind the x transfers (bandwidth is the
        # gate early on; these aren't needed until mid-block)
        for c in range(4, NCH):
            cs = slice(c * CH, (c + 1) * CH)
            nc.scalar.dma_start(qTt[64:128, cs], bsynT[:, cs])
            nc.scalar.dma_start(kTt[64:128, cs], onehotT[:, cs])

        with (
            tc.tile_pool(name="psB", bufs=3, space="PSUM") as psB,
            tc.tile_pool(name="psAcc", bufs=1, space="PSUM") as psAcc,
            tc.tile_pool(name="psX", bufs=1, space="PSUM") as psX,
        ):
            # ---- helpers ----
            def emit_qk(c, pool, on_act=False):
                cs = slice(c * CH, (c + 1) * CH)
                qkp = pool.tile([128, CH], F32, name="qkp",
                                tag="oacc" if pool is psAcc else "px")
                mm(qkp[:], wqk_sb[:, 0:128], xc0[c][:], start=True, stop=False)
                mm(qkp[:], wqk_sb[:, 128:256], xc1[c][:], start=False,
                   stop=True)
                nc.vector.tensor_scalar_add(qTt[0:D, cs], qkp[0:D, :],
                                            b2_sb[0:D, 0:1])
                nc.vector.tensor_scalar_add(kTt[0:D, cs], qkp[D:128, :],
                                            b2_sb[D:128, 0:1])

            vts_tiles = {}

            def emit_v_proj(c):
                cs = slice(c * CH, (c + 1) * CH)
                vtp = psX.tile([DV, CH], F32, name="vtp", tag="px")
                mm(vtp[:], wv_sb[:, 0:DV], xc0[c][:], start=True, stop=False)
                mm(vtp[:], wv_sb[:, DV:2 * DV], xc1[c][:], start=False,
                   stop=True)
                # bias column [bv | 1 | 0..] adds the ones row during eviction
                vts = vtsp.tile([DV, CH], F32R, name="vts", tag="vts")
                nc.vector.tensor_scalar_add(vts[:], vtp[:], b2_sb[0:DV, 1:2])
                vts_tiles[c] = vts

            def emit_v_flip(c):
                vts = vts_tiles.pop(c)
                vtr = psX.tile([KT, 4 * DV], F32R, name="vtr", tag="px")
                for m in range(4):
                    nc.tensor.transpose(vtr[:, m * DV:(m + 1) * DV],
                                        vts[:, m * KT:(m + 1) * KT],
                                        id_sb[0:DV, 0:DV])
                nc.vector.tensor_copy(vb[:, 4 * c * DV:(4 * c + 4) * DV],
                                      vtr[:])

            oaccs = {}

            def proj_steps(qb):
                """Deferred output projection for query block qb; the oacc
                eviction happens immediately (DVE is idle), the PE matmuls
                run later in loose slots of the next block."""
                qsl = slice(qb * QB, (qb + 1) * QB)
                oacc = oaccs.pop(qb)
                nc.vector.tensor_copy(oall[:, qsl], oacc[0:D, :])
                nc.vector.tensor_copy(l_sb[:, qsl], oacc[LCOL:LCOL + 1, :])

                def s1():
                    pj = psB.tile([128, QB], F32, name="pj", tag="s3")
                    ob = obp.tile([128, QB], F16, name="ob", tag="ob")
                    mm(pj[:], wo_sb[:, 0:128], oall[:, qsl],
                       start=True, stop=True)
                    nc.vector.tensor_copy(ob[:], pj[:])
                    nc.gpsimd.dma_start(outT[0:128, qsl], ob[:])

                def s2():
                    pj = psB.tile([128, QB], F32, name="pj", tag="s3")
                    ob = obp.tile([128, QB], F16, name="ob", tag="ob")
                    mm(pj[:], wo_sb[:, 128:256], oall[:, qsl],
                       start=True, stop=True)
                    nc.vector.tensor_copy(ob[:], pj[:])
                    nc.gpsimd.dma_start(outT[128:256, qsl], ob[:])
                    nc.gpsimd.dma_start(lT[:, qsl], l_sb[:, qsl])

                return [s1, s2]

            def proj_last(qb):
                """Final block: same halves, but the output DMA is split
                across four idle queues so the tail transfer parallelizes."""
                qsl = slice(qb * QB, (qb + 1) * QB)
                oacc = oaccs.pop(qb)
                nc.vector.tensor_copy(oall[:, qsl], oacc[0:D, :])
                nc.vector.tensor_copy(l_sb[:, qsl], oacc[LCOL:LCOL + 1, :])
                nc.gpsimd.dma_start(lT[:, qsl], l_sb[:, qsl])
                for half, ofs in ((0, 0), (1, 128)):
                    pj = psB.tile([128, QB], F32, name="pjl", tag="s3")
                    ob = obp.tile([128, QB], F16, name="obl", tag="ob")
                    mm(pj[:], wo_sb[:, ofs:ofs + 128], oall[:, qsl],
                       start=True, stop=True)
                    nc.vector.tensor_copy(ob[:], pj[:])
                    q0 = qb * QB
                    eng = (nc.sync, nc.scalar) if half == 0 else \
                          (nc.gpsimd, nc.sync)
                    eng[0].dma_start(outT[ofs:ofs + 128, q0:q0 + 256],
                                     ob[:, 0:256])
                    eng[1].dma_start(outT[ofs:ofs + 128, q0 + 256:q0 + 512],
                                     ob[:, 256:512])

            # PE p-state warmup: ~20 dummy matmuls on the first-arrived
            # weights keep the array streaming while the x DMA lands, so
            # the real stream starts at full clock instead of ramping
            # through its first dozen groups.
            warm = psX.tile([128, 256], F32, name="warm", tag="px")
            for _ in range(20):
                mm(warm[:], wqk_sb[:, 0:128], wqk_sb[:], start=True, stop=True)

            # ---- pre-stream: minimum to start group 0; the rest of the
            # projections/flips are injected into the qb-0 stream. PV lags
            # the score stream by PVLAG groups, which relaxes every v-chain
            # deadline and frees the oacc PSUM slot for qk3/qk4 early on.
            emit_qk(0, psX)
            emit_qk(1, psAcc)
            emit_v_proj(0)          # psX slot
            emit_v_flip(0)          # psX slot
            emit_qk(2, psX)

            inject = {
                0: [lambda: emit_v_proj(1)],
                1: [lambda: emit_v_flip(1), lambda: emit_qk(3, psAcc)],
                2: [lambda: emit_v_proj(2)],
                3: [lambda: emit_v_flip(2), lambda: emit_qk(4, psAcc)],
                4: [lambda: emit_qk(5, psX)],
                5: [lambda: emit_v_proj(3)],
                6: [lambda: emit_v_flip(3)],
                7: [lambda: emit_qk(6, psX)],
                8: [lambda: emit_v_proj(4)],
                9: [lambda: emit_v_flip(4)],
                10: [lambda: emit_qk(7, psX)],
                11: [lambda: emit_v_proj(5)],
                12: [lambda: emit_v_flip(5)],
                13: [lambda: emit_v_proj(6)],
                14: [lambda: emit_v_flip(6)],
                15: [lambda: emit_v_proj(7)],
            }
            inject_qb1 = {0: [lambda: emit_v_flip(7)]}

            # ---- attention stream (PV lags scores by PVLAG groups) ----
            PVLAG = 6
            pv_queue = []
            pending_proj = []

            def emit_pv(qb, gi, p3):
                if gi == 0:
                    oaccs[qb] = psAcc.tile([DV, QB], F32, name="oacc",
                                           tag="oacc")
                oacc = oaccs[qb]
                for m in range(GRP):
                    j = GRP * gi + m
                    mm(oacc[:], vb[:, j * DV:(j + 1) * DV],
                       p3[:, m * QB:(m + 1) * QB],
                       start=(j == 0), stop=(j == NKT - 1))

            done_qb = {}

            def drain_one_pv():
                qb0_, gi0_, p30_ = pv_queue.pop(0)
                emit_pv(qb0_, gi0_, p30_)
                if gi0_ == NG - 1:
                    done_qb[qb0_] = True

            for qb in range(NQB):
                qsl = slice(qb * QB, (qb + 1) * QB)
                for gi in range(NG):
                    if qb == 0:
                        for thunk in inject.get(gi, ()):
                            thunk()
                    elif qb == 1:
                        for thunk in inject_qb1.get(gi, ()):
                            thunk()
                    s3 = psB.tile([128, GRP * QB], F32, name="s3", tag="s3")
                    for m in range(GRP):
                        j = GRP * gi + m
                        jl = slice(j * KT, (j + 1) * KT)
                        mm(s3[:, m * QB:(m + 1) * QB], kTt[:, jl], qTt[:, qsl],
                           start=True, stop=True)
                    p3 = ptp.tile([128, GRP * QB], F16, name="p3", tag="p3")
                    nc.scalar.activation(p3[:], s3[:], Exp)
                    pv_queue.append((qb, gi, p3))
                    if len(pv_queue) > PVLAG:
                        drain_one_pv()
                    # previous block done accumulating? evict + start proj
                    if done_qb.pop(qb - 1, None):
                        pending_proj = proj_steps(qb - 1)
                    if pending_proj and gi % 6 == 5:
                        pending_proj.pop(0)()
            while pv_queue:
                drain_one_pv()
            for step in pending_proj:
                step()
            proj_last(NQB - 1)


_NC_CACHE = {}


def _get_program():
    if "nc" not in _NC_CACHE:
        _NC_CACHE["nc"] = build_program()
    return _NC_CACHE["nc"]


def make_in_maps(x, codons, syn_bias, wq, bq, wk, bk, wv, bv, wo):
    in_maps = []
    for core in range(8):
        b, h = divmod(core, NH)
        hsl = slice(h * D, (h + 1) * D)
        cod = codons[b]
        onehotT = np.zeros((D, S), np.float16)
        onehotT[cod, np.arange(S)] = 1.0
        # [wqT/8 | wkT] packed as [hi-half ; lo-half] -> [128, 256]
        wqk_full = np.concatenate([wq[hsl, :].T / 8.0, wk[hsl, :].T], axis=1)
        wqk = np.concatenate([wqk_full[0:128], wqk_full[128:256]], axis=1)
        wvp = np.concatenate(
            [wv[hsl, :].T, np.zeros((HID, DV - D), np.float32)], axis=1)
        wv2 = np.concatenate([wvp[0:128], wvp[128:256]], axis=1)
        bias2 = np.zeros((128, 2), np.float32)
        bias2[:, 0] = np.concatenate([bq[hsl] / 8.0, bk[hsl]])
        bias2[0:D, 1] = bv[hsl]
        bias2[LCOL, 1] = 1.0
        in_maps.append({
            "xT": x[b].T.astype(np.float16),
            "wqk": wqk.astype(np.float16),
            "wv2": wv2.astype(np.float16),
            "bias2": bias2,
            "bsynT": np.ascontiguousarray(syn_bias.T[:, cod]).astype(np.float16),
            "onehotT": onehotT,
            "woT": wo[:, hsl].T.astype(np.float16),
            "idm": np.eye(128, dtype=np.float32),
        })
    return in_maps


def kernel_run(inputs, trace=False):
    x = np.asarray(inputs["x"], np.float32)
    codons = np.asarray(inputs["codons"]).astype(np.int64)
    syn_bias = np.asarray(inputs["syn_bias"], np.float32)
    wq = np.asarray(inputs["wq"], np.float32)
    bq = np.asarray(inputs["bq"], np.float32)
    wk = np.asarray(inputs["wk"], np.float32)
    bk = np.asarray(inputs["bk"], np.float32)
    wv = np.asarray(inputs["wv"], np.float32)
    bv = np.asarray(inputs["bv"], np.float32)
    wo = np.asarray(inputs["wo"], np.float32)
    bo = np.asarray(inputs["bo"], np.float32)

    nc = _get_program()
    in_maps = make_in_maps(x, codons, syn_bias, wq, bq, wk, bk, wv, bv, wo)
    res = run_bass_kernel_spmd(nc, in_maps, core_ids=list(range(8)), trace=trace)

    out = np.empty((B, S, HID), np.float32)
    for b in range(B):
        acc = None
        for h in range(NH):
            r = res.results[NH * b + h]
            part = r["outT"].astype(np.float32) / r["lT"]   # normalize per head
            acc = part if acc is None else acc + part
        out[b] = acc.T + bo
    return out, res


def kernel(**inputs):
    out, _ = kernel_run(inputs, trace=False)
    return out



# revision 5
# speedup vs baseline: 1.0009x; 1.0009x over previous
"""CodonAttention Trainium2 kernel (fp16 stream, issue-lean pipeline).

Math (per batch b, head h):
  q = x @ wq.T + bq ; k = x @ wk.T + bk ; v = x @ wv.T + bv   (head slices)
  scores = q k^T / 8 + syn_bias[codons_i, codons_j]
  out    = softmax(scores) @ v ;  final = concat_heads(out) @ wo.T + bo

Key algebraic trick: the pairwise codon bias factors through one-hots,
  pair_bias = onehot @ syn_bias @ onehot.T
so augmenting q' = [(q+bq)/8 | onehot @ syn_bias] and k' = [k | onehot] gives
  scores = q' @ k'.T        (effective head dim 128 — exactly one partition)
Softmax runs without max-subtraction (|scores| <= ~4.3, exp safe in fp32) and
the row-sum l is obtained with a ones-column in v: [O | l] = P @ [v | 1].

Sharding: 8 cores = (batch b in {0,1}) x (head h in {0..3}). Each core runs
the full attention for its (b, h), producing the UNNORMALIZED partial
projection outT = (wo_h @ O_h.T) (256, 4096) plus denominators lT (1, 4096);
the host divides, sums the 4 head partials per batch, transposes, adds bo.

Profile-driven design (trace facts from this hardware):
- Phase B is ACT-bound: exp runs 1 col/cycle @1.2GHz regardless of dtype,
  ~1.0us per [128,1024] group; the PE streams 512-row fp16 matmuls at
  ~0.42 ns/row so 4 matmuls/group (~0.87us) fit under the exp.
- Every dma_start costs ~0.6-1us of *issue* time on its queue, so DMA
  issues are spread: Sync + Scalar queues carry x/weights (hwdge),
  GpSimd carries the bias streams and all output DMAs (swdge, idle
  engine). Queue order puts chunk 0 first so compute starts ~3us in.
- Engines execute their queue in order, so late-chunk work must not sit
  in front of the attention stream: q/k projections and v transposes
  for chunks 3..7 are injected INTO the qb-0 attention stream right
  before the groups that consume them.
- The per-block output projection is deferred two groups into the next
  query block so its oacc->oall->PE chain never stalls the score
  pipeline (it runs in loose slots, PSUM bank shared with the qk
  projection pool).
- PSUM budget (8 banks): scores double-buffer 2x2 + oacc 2 + v-flip 1 +
  qkproj/outproj shared 1.
"""

import numpy as np

import concourse.mybir as mybir
import concourse.tile as tile
from concourse import bacc
from concourse.bass_utils import run_bass_kernel_spmd


def _ensure_axon_ntff_hook():
    """This image's antenv package lacks axon_hooks, so
    run_bass_kernel_spmd(trace=True) (or BASS_TRACE=1) would die on the
    import. Register a compatible module backed by the libaxon_pjrt C ABI
    so tracing works if a caller requests it."""
    import sys
    try:
        import antenv.axon_hooks  # noqa: F401
        return
    except ImportError:
        pass
    import contextlib
    import ctypes
    import types
    try:
        lib = ctypes.CDLL("/opt/axon/libaxon_pjrt.so")
        has = hasattr(lib, "axon_start_nrt_profile")
    except OSError:
        has = False
    if has:
        lib.axon_start_nrt_profile.argtypes = [ctypes.POINTER(ctypes.c_int64),
                                               ctypes.c_size_t]
        lib.axon_start_nrt_profile.restype = ctypes.c_int64
        lib.axon_stop_nrt_profile.argtypes = [ctypes.c_char_p]
        lib.axon_stop_nrt_profile.restype = ctypes.c_int64

        @contextlib.contextmanager
        def _hook(output_dir, device_ids):
            import jax
            jax.devices()
            if device_ids:
                ids = (ctypes.c_int64 * len(device_ids))(*device_ids)
                rc = lib.axon_start_nrt_profile(ids, len(device_ids))
            else:
                rc = lib.axon_start_nrt_profile(None, 0)
            if rc != 0:
                raise RuntimeError(f"axon_start_nrt_profile rc={rc}")
            try:
                yield
            finally:
                lib.axon_stop_nrt_profile(str(output_dir).encode())
    else:
        _hook = None

    mod = types.ModuleType("antenv.axon_hooks")
    _state = {"hook": _hook}
    mod.get_axon_ntff_profile_hook = lambda: _state["hook"]
    mod.set_axon_ntff_profile_hook = lambda h: _state.__setitem__("hook", h)
    sys.modules["antenv.axon_hooks"] = mod


_ensure_axon_ntff_hook()

B, S, HID, NH, D = 2, 4096, 256, 4, 64
DV = D + 4         # v cols + ones column + 3 pad
LCOL = D           # index of the ones column inside a v tile
QB = 512           # query block (free dim of score matmuls)
KT = 128           # key tile (partition dim of transposed scores)
CH = 512           # x chunk width
NCH = S // CH      # 8
NQB = S // QB      # 8
NKT = S // KT      # 32
GRP = 2            # key tiles per exp group (2 PSUM banks per group)
NG = NKT // GRP    # 16 groups per query block

F32 = mybir.dt.float32
F32R = mybir.dt.float32r
F16 = mybir.dt.float16
Exp = mybir.ActivationFunctionType.Exp


def build_program():
    nc = bacc.Bacc("TRN2", target_bir_lowering=False, debug=False, num_devices=8)

    def di(name, shape, dt=F16):
        return nc.dram_tensor(name, shape, dt, kind="ExternalInput").ap()

    xT = di("xT", [HID, S])            # x[b].T
    wqk = di("wqk", [128, 256])        # [wqT_hi/8|wkT_hi ; wqT_lo/8|wkT_lo]
    wv2 = di("wv2", [128, 2 * DV])     # [wvT_hi | wvT_lo], col 64.. pad 0
    bias2 = di("bias2", [128, 2], F32) # col0 = [bq/8; bk], col1 = [bv;1;0..]
    bsynT = di("bsynT", [D, S])        # (onehot @ syn_bias).T
    onehotT = di("onehotT", [D, S])
    woT = di("woT", [D, HID])          # wo[:, hslice].T
    idm = di("idm", [128, 128], F32R)  # identity for TensorE transpose
    outT = nc.dram_tensor("outT", [HID, S], F16, kind="ExternalOutput").ap()
    lT = nc.dram_tensor("lT", [1, S], F32, kind="ExternalOutput").ap()

    with tile.TileContext(nc) as tc:
        _body(tc, xT, wqk, wv2, bias2, bsynT, onehotT, woT, idm, outT, lT)
    nc.compile()
    return nc


def _body(tc, xT, wqk, wv2, bias2, bsynT, onehotT, woT, idm, outT, lT):
    nc = tc.nc
    mm = nc.tensor.matmul

    with (
        tc.tile_pool(name="const", bufs=1) as constp,
        tc.tile_pool(name="big", bufs=1) as bigp,
        tc.tile_pool(name="vts", bufs=2) as vtsp,
        tc.tile_pool(name="pt", bufs=9) as ptp,
        tc.tile_pool(name="ob", bufs=2) as obp,
    ):
        # ---- constants ----
        wqk_sb = constp.tile([128, 256], F16, name="wqk_sb", tag="wqk_sb")
        wv_sb = constp.tile([128, 2 * DV], F16, name="wv_sb", tag="wv_sb")
        b2_sb = constp.tile([128, 2], F32, name="b2_sb", tag="b2_sb")
        wo_sb = constp.tile([D, HID], F16, name="wo_sb", tag="wo_sb")
        id_sb = constp.tile([128, 128], F32R, name="id_sb", tag="id_sb")
        scr = constp.tile([1, 1], F32, name="scr", tag="scr")

        # persistent activations (subregion deps make slices per-chunk)
        xc0 = [bigp.tile([128, CH], F16, name=f"xc0_{c}", tag=f"xc0_{c}")
               for c in range(NCH)]
        xc1 = [bigp.tile([128, CH], F16, name=f"xc1_{c}", tag=f"xc1_{c}")
               for c in range(NCH)]
        qTt = bigp.tile([128, S], F16, name="qTt", tag="qTt")  # 0:64 q/8, 64:128 bsynT
        kTt = bigp.tile([128, S], F16, name="kTt", tag="kTt")  # 0:64 k,   64:128 onehotT
        vb = bigp.tile([128, NKT * DV], F16, name="vb", tag="vb")  # v' key-major
        oall = bigp.tile([D, S], F16, name="oall", tag="oall")
        l_sb = bigp.tile([1, S], F32, name="l_sb", tag="l_sb")

        # ---- DMA issues. The scalar queue carries ZERO dma work: every
        # issue there (~0.6us each) serializes with the ACTIVATE stream,
        # which is the kernel's bottleneck engine. Sync (hwdge) takes the
        # weights + x chunks; GpSimd (swdge) takes the bias streams and,
        # later, all output DMAs.
        nc.sync.dma_start(wqk_sb[:], wqk[:])
        nc.sync.dma_start(xc0[0][:], xT[0:128, 0:CH])
        nc.sync.dma_start(xc1[0][:], xT[128:256, 0:CH])
        nc.sync.dma_start(b2_sb[:], bias2[:])
        nc.sync.dma_start(wv_sb[:], wv2[:])
        nc.sync.dma_start(id_sb[:], idm[:])
        for c in range(1, NCH):
            cs = slice(c * CH, (c + 1) * CH)
            nc.sync.dma_start(xc0[c][:], xT[0:128, cs])
            nc.sync.dma_start(xc1[c][:], xT[128:256, cs])
        for c in range(NCH):
            cs = slice(c * CH, (c + 1) * CH)
            nc.gpsimd.dma_start(qTt[64:128, cs], bsynT[:, cs])
            nc.gpsimd.dma_start(kTt[64:128, cs], onehotT[:, cs])
        nc.gpsimd.dma_start(wo_sb[:], woT[:])

        # warm the ACT exp table (~2.7us) while projections run
        nc.scalar.activation(scr[:], b2_sb[0:1, 0:1], Exp)

        with (
            tc.tile_pool(name="psB", bufs=3, space="PSUM") as psB,
            tc.tile_pool(name="psAcc", bufs=1, space="PSUM") as psAcc,
            tc.tile_pool(name="psX", bufs=1, space="PSUM") as psX,
        ):
            # ---- helpers ----
            def emit_qk(c, pool, on_act=False):
                cs = slice(c * CH, (c + 1) * CH)
                qkp = pool.tile([128, CH], F32, name="qkp",
                                tag="oacc" if pool is psAcc else "px")
                mm(qkp[:], wqk_sb[:, 0:128], xc0[c][:], start=True, stop=False)
                mm(qkp[:], wqk_sb[:, 128:256], xc1[c][:], start=False,
                   stop=True)
                nc.vector.tensor_scalar_add(qTt[0:D, cs], qkp[0:D, :],
                                            b2_sb[0:D, 0:1])
                nc.vector.tensor_scalar_add(kTt[0:D, cs], qkp[D:128, :],
                                            b2_sb[D:128, 0:1])

            vts_tiles = {}

            def emit_v_proj(c):
                cs = slice(c * CH, (c + 1) * CH)
                vtp = psX.tile([DV, CH], F32, name="vtp", tag="px")
                mm(vtp[:], wv_sb[:, 0:DV], xc0[c][:], start=True, stop=False)
                mm(vtp[:], wv_sb[:, DV:2 * DV], xc1[c][:], start=False,
                   stop=True)
                # bias column [bv | 1 | 0..] adds the ones row during eviction
                vts = vtsp.tile([DV, CH], F32R, name="vts", tag="vts")
                nc.vector.tensor_scalar_add(vts[:], vtp[:], b2_sb[0:DV, 1:2])
                vts_tiles[c] = vts

            def emit_v_flip(c):
                vts = vts_tiles.pop(c)
                vtr = psX.tile([KT, 4 * DV], F32R, name="vtr", tag="px")
                for m in range(4):
                    nc.tensor.transpose(vtr[:, m * DV:(m + 1) * DV],
                                        vts[:, m * KT:(m + 1) * KT],
                                        id_sb[0:DV, 0:DV])
                nc.vector.tensor_copy(vb[:, 4 * c * DV:(4 * c + 4) * DV],
                                      vtr[:])

            oaccs = {}

            def proj_steps(qb):
                """Deferred output projection for query block qb; the oacc
                eviction happens immediately (DVE is idle), the PE matmuls
                run later in loose slots of the next block."""
                qsl = slice(qb * QB, (qb + 1) * QB)
                oacc = oaccs.pop(qb)
                nc.vector.tensor_copy(oall[:, qsl], oacc[0:D, :])
                nc.vector.tensor_copy(l_sb[:, qsl], oacc[LCOL:LCOL + 1, :])

                def s1():
                    pj = psB.tile([128, QB], F32, name="pj", tag="s3")
                    ob = obp.tile([128, QB], F16, name="ob", tag="ob")
                    mm(pj[:], wo_sb[:, 0:128], oall[:, qsl],
                       start=True, stop=True)
                    nc.vector.tensor_copy(ob[:], pj[:])
                    nc.gpsimd.dma_start(outT[0:128, qsl], ob[:])

                def s2():
                    pj = psB.tile([128, QB], F32, name="pj", tag="s3")
                    ob = obp.tile([128, QB], F16, name="ob", tag="ob")
                    mm(pj[:], wo_sb[:, 128:256], oall[:, qsl],
                       start=True, stop=True)
                    nc.vector.tensor_copy(ob[:], pj[:])
                    nc.gpsimd.dma_start(outT[128:256, qsl], ob[:])
                    nc.gpsimd.dma_start(lT[:, qsl], l_sb[:, qsl])

                return [s1, s2]

            def proj_last(qb):
                """Final block: same halves, but the output DMA is split
                across four idle queues so the tail transfer parallelizes."""
                qsl = slice(qb * QB, (qb + 1) * QB)
                oacc = oaccs.pop(qb)
                nc.vector.tensor_copy(oall[:, qsl], oacc[0:D, :])
                nc.vector.tensor_copy(l_sb[:, qsl], oacc[LCOL:LCOL + 1, :])
                nc.gpsimd.dma_start(lT[:, qsl], l_sb[:, qsl])
                for half, ofs in ((0, 0), (1, 128)):
                    pj = psB.tile([128, QB], F32, name="pjl", tag="s3")
                    ob = obp.tile([128, QB], F16, name="obl", tag="ob")
                    mm(pj[:], wo_sb[:, ofs:ofs + 128], oall[:, qsl],
                       start=True, stop=True)
                    nc.vector.tensor_copy(ob[:], pj[:])
                    q0 = qb * QB
                    eng = (nc.sync, nc.scalar) if half == 0 else \
                          (nc.gpsimd, nc.sync)
                    eng[0].dma_start(outT[ofs:ofs + 128, q0:q0 + 256],
                                     ob[:, 0:256])
                    eng[1].dma_start(outT[ofs:ofs + 128, q0 + 256:q0 + 512],
                                     ob[:, 256:512])

            # PE p-state warmup: dummy matmuls on the first-arrived weights
            # bridge the gap until the x chunk-0 DMA lands (so qk0 doesn't
            # run at the cold 0.65GHz p-state). 10 is sized so the PE never
            # idles between warmup end and xc0 arrival.
            warm = psX.tile([128, 256], F32, name="warm", tag="px")
            for _ in range(10):
                mm(warm[:], wqk_sb[:, 0:128], wqk_sb[:], start=True, stop=True)

            # ---- pre-stream: ONLY the chunk-0 q/k projection — kTt chunk 0
            # covers key tiles 0..3 (groups 0-1), so exp group 0 can start
            # ~1us after xc0 lands. Everything else (qk 1..7, v proj/flips)
            # is injected into the group loop just ahead of its deadline:
            # kTt chunk c feeds groups 2c..2c+1 -> qk(c) at group 2c-1;
            # vb chunk c is read by PV(g) for g in {2c..2c+1} which drains
            # at group g+PVLAG -> vflip(c) by group 2c+PVLAG-1.
            emit_qk(0, psX)

            inject = {
                1: [lambda: emit_qk(1, psAcc)],
                2: [lambda: emit_v_proj(0)],
                3: [lambda: emit_qk(2, psX)],
                4: [lambda: emit_v_flip(0)],
                5: [lambda: emit_qk(3, psAcc)],
                6: [lambda: emit_v_proj(1)],
                7: [lambda: emit_v_flip(1), lambda: emit_qk(4, psX)],
                8: [lambda: emit_v_proj(2)],
                9: [lambda: emit_v_flip(2), lambda: emit_qk(5, psX)],
                10: [lambda: emit_v_proj(3)],
                11: [lambda: emit_v_flip(3), lambda: emit_qk(6, psX)],
                12: [lambda: emit_v_proj(4)],
                13: [lambda: emit_v_flip(4), lambda: emit_qk(7, psX)],
                14: [lambda: emit_v_proj(5)],
                15: [lambda: emit_v_flip(5)],
            }
            inject_qb1 = {
                0: [lambda: emit_v_proj(6)],
                1: [lambda: emit_v_flip(6)],
                2: [lambda: emit_v_proj(7)],
                3: [lambda: emit_v_flip(7)],
            }

            # ---- attention stream (PV lags scores by PVLAG groups) ----
            PVLAG = 6
            pv_queue = []
            pending_proj = []

            def emit_pv(qb, gi, p3):
                if gi == 0:
                    oaccs[qb] = psAcc.tile([DV, QB], F32, name="oacc",
                                           tag="oacc")
                oacc = oaccs[qb]
                for m in range(GRP):
                    j = GRP * gi + m
                    mm(oacc[:], vb[:, j * DV:(j + 1) * DV],
                       p3[:, m * QB:(m + 1) * QB],
                       start=(j == 0), stop=(j == NKT - 1))

            done_qb = {}

            def drain_one_pv():
                qb0_, gi0_, p30_ = pv_queue.pop(0)
                emit_pv(qb0_, gi0_, p30_)
                if gi0_ == NG - 1:
                    done_qb[qb0_] = True

            for qb in range(NQB):
                qsl = slice(qb * QB, (qb + 1) * QB)
                for gi in range(NG):
                    if qb == 0:
                        for thunk in inject.get(gi, ()):
                            thunk()
                    elif qb == 1:
                        for thunk in inject_qb1.get(gi, ()):
                            thunk()
                    s3 = psB.tile([128, GRP * QB], F32, name="s3", tag="s3")
                    for m in range(GRP):
                        j = GRP * gi + m
                        jl = slice(j * KT, (j + 1) * KT)
                        mm(s3[:, m * QB:(m + 1) * QB], kTt[:, jl], qTt[:, qsl],
                           start=True, stop=True)
                    p3 = ptp.tile([128, GRP * QB], F16, name="p3", tag="p3")
                    nc.scalar.activation(p3[:], s3[:], Exp)
                    pv_queue.append((qb, gi, p3))
                    # Last block: taper the PV lag down to 1 (2 drains per
                    # group) so the post-stream tail is one PV group instead
                    # of PVLAG+1. PE slack per group (~580ns) absorbs the
                    # extra PV matmuls.
                    lag = 1 if qb == NQB - 1 else PVLAG
                    drains = 0
                    while len(pv_queue) > lag and drains < 2:
                        drain_one_pv()
                        drains += 1
                    # previous block done accumulating? evict + start proj
                    if done_qb.pop(qb - 1, None):
                        pending_proj = proj_steps(qb - 1)
                    if pending_proj and gi % 6 == 5:
                        pending_proj.pop(0)()
            while pv_queue:
                drain_one_pv()
            for step in pending_proj:
                step()
            proj_last(NQB - 1)


_NC_CACHE = {}


def _get_program():
    if "nc" not in _NC_CACHE:
        _NC_CACHE["nc"] = build_program()
    return _NC_CACHE["nc"]


def make_in_maps(x, codons, syn_bias, wq, bq, wk, bk, wv, bv, wo):
    in_maps = []
    for core in range(8):
        b, h = divmod(core, NH)
        hsl = slice(h * D, (h + 1) * D)
        cod = codons[b]
        onehotT = np.zeros((D, S), np.float16)
        onehotT[cod, np.arange(S)] = 1.0
        # [wqT/8 | wkT] packed as [hi-half ; lo-half] -> [128, 256]
        wqk_full = np.concatenate([wq[hsl, :].T / 8.0, wk[hsl, :].T], axis=1)
        wqk = np.concatenate([wqk_full[0:128], wqk_full[128:256]], axis=1)
        wvp = np.concatenate(
            [wv[hsl, :].T, np.zeros((HID, DV - D), np.float32)], axis=1)
        wv2 = np.concatenate([wvp[0:128], wvp[128:256]], axis=1)
        bias2 = np.zeros((128, 2), np.float32)
        bias2[:, 0] = np.concatenate([bq[hsl] / 8.0, bk[hsl]])
        bias2[0:D, 1] = bv[hsl]
        bias2[LCOL, 1] = 1.0
        in_maps.append({
            "xT": x[b].T.astype(np.float16),
            "wqk": wqk.astype(np.float16),
            "wv2": wv2.astype(np.float16),
            "bias2": bias2,
            "bsynT": np.ascontiguousarray(syn_bias.T[:, cod]).astype(np.float16),
            "onehotT": onehotT,
            "woT": wo[:, hsl].T.astype(np.float16),
            "idm": np.eye(128, dtype=np.float32),
        })
    return in_maps


def kernel_run(inputs, trace=False):
    x = np.asarray(inputs["x"], np.float32)
    codons = np.asarray(inputs["codons"]).astype(np.int64)
    syn_bias = np.asarray(inputs["syn_bias"], np.float32)
    wq = np.asarray(inputs["wq"], np.float32)
    bq = np.asarray(inputs["bq"], np.float32)
    wk = np.asarray(inputs["wk"], np.float32)
    bk = np.asarray(inputs["bk"], np.float32)
    wv = np.asarray(inputs["wv"], np.float32)
    bv = np.asarray(inputs["bv"], np.float32)
    wo = np.asarray(inputs["wo"], np.float32)
    bo = np.asarray(inputs["bo"], np.float32)

    nc = _get_program()
    in_maps = make_in_maps(x, codons, syn_bias, wq, bq, wk, bk, wv, bv, wo)
    res = run_bass_kernel_spmd(nc, in_maps, core_ids=list(range(8)), trace=trace)

    out = np.empty((B, S, HID), np.float32)
    for b in range(B):
        acc = None
        for h in range(NH):
            r = res.results[NH * b + h]
            part = r["outT"].astype(np.float32) / r["lT"]   # normalize per head
            acc = part if acc is None else acc + part
        out[b] = acc.T + bo
    return out, res


def kernel(**inputs):
    out, _ = kernel_run(inputs, trace=False)
    return out



# revision 17
# speedup vs baseline: 1.0295x; 1.0286x over previous
"""CodonAttention Trainium2 kernel (fp16 stream, issue-lean pipeline).

Math (per batch b, head h):
  q = x @ wq.T + bq ; k = x @ wk.T + bk ; v = x @ wv.T + bv   (head slices)
  scores = q k^T / 8 + syn_bias[codons_i, codons_j]
  out    = softmax(scores) @ v ;  final = concat_heads(out) @ wo.T + bo

Key algebraic trick: the pairwise codon bias factors through one-hots,
  pair_bias = onehot @ syn_bias @ onehot.T
so augmenting q' = [(q+bq)/8 | onehot @ syn_bias] and k' = [k | onehot] gives
  scores = q' @ k'.T        (effective head dim 128 — exactly one partition)
Softmax runs without max-subtraction (|scores| <= ~4.3, exp safe in fp32) and
the row-sum l is obtained with a ones-column in v: [O | l] = P @ [v | 1].

Sharding: 8 cores = (batch b in {0,1}) x (head h in {0..3}). Each core runs
the full attention for its (b, h), producing the UNNORMALIZED partial
projection outT = (wo_h @ O_h.T) (256, 4096) plus denominators lT (1, 4096);
the host divides, sums the 4 head partials per batch, transposes, adds bo.

Profile-driven design (trace facts from this hardware):
- Phase B is ACT-bound: exp runs 1 col/cycle @1.2GHz regardless of dtype,
  ~1.0us per [128,1024] group; the PE streams 512-row fp16 matmuls at
  ~0.42 ns/row so 4 matmuls/group (~0.87us) fit under the exp.
- Every dma_start costs ~0.6-1us of *issue* time on its queue, so DMA
  issues are spread: Sync + Scalar queues carry x/weights (hwdge),
  GpSimd carries the bias streams and all output DMAs (swdge, idle
  engine). Queue order puts chunk 0 first so compute starts ~3us in.
- Engines execute their queue in order, so late-chunk work must not sit
  in front of the attention stream: q/k projections and v transposes
  for chunks 3..7 are injected INTO the qb-0 attention stream right
  before the groups that consume them.
- The per-block output projection is deferred two groups into the next
  query block so its oacc->oall->PE chain never stalls the score
  pipeline (it runs in loose slots, PSUM bank shared with the qk
  projection pool).
- PSUM budget (8 banks): scores double-buffer 2x2 + oacc 2 + v-flip 1 +
  qkproj/outproj shared 1.
"""

import numpy as np

import concourse.mybir as mybir
import concourse.tile as tile
from concourse import bacc
from concourse.bass_utils import run_bass_kernel_spmd


def _ensure_axon_ntff_hook():
    """This image's antenv package lacks axon_hooks, so
    run_bass_kernel_spmd(trace=True) (or BASS_TRACE=1) would die on the
    import. Register a compatible module backed by the libaxon_pjrt C ABI
    so tracing works if a caller requests it."""
    import sys
    try:
        import antenv.axon_hooks  # noqa: F401
        return
    except ImportError:
        pass
    import contextlib
    import ctypes
    import types
    try:
        lib = ctypes.CDLL("/opt/axon/libaxon_pjrt.so")
        has = hasattr(lib, "axon_start_nrt_profile")
    except OSError:
        has = False
    if has:
        lib.axon_start_nrt_profile.argtypes = [ctypes.POINTER(ctypes.c_int64),
                                               ctypes.c_size_t]
        lib.axon_start_nrt_profile.restype = ctypes.c_int64
        lib.axon_stop_nrt_profile.argtypes = [ctypes.c_char_p]
        lib.axon_stop_nrt_profile.restype = ctypes.c_int64

        @contextlib.contextmanager
        def _hook(output_dir, device_ids):
            import jax
            jax.devices()
            if device_ids:
                ids = (ctypes.c_int64 * len(device_ids))(*device_ids)
                rc = lib.axon_start_nrt_profile(ids, len(device_ids))
            else:
                rc = lib.axon_start_nrt_profile(None, 0)
            if rc != 0:
                raise RuntimeError(f"axon_start_nrt_profile rc={rc}")
            try:
                yield
            finally:
                lib.axon_stop_nrt_profile(str(output_dir).encode())
    else:
        _hook = None

    mod = types.ModuleType("antenv.axon_hooks")
    _state = {"hook": _hook}
    mod.get_axon_ntff_profile_hook = lambda: _state["hook"]
    mod.set_axon_ntff_profile_hook = lambda h: _state.__setitem__("hook", h)
    sys.modules["antenv.axon_hooks"] = mod


_ensure_axon_ntff_hook()

B, S, HID, NH, D = 2, 4096, 256, 4, 64
DV = D + 4         # v cols + ones column + 3 pad
LCOL = D           # index of the ones column inside a v tile
QB = 512           # query block (free dim of score matmuls)
KT = 128           # key tile (partition dim of transposed scores)
CH = 512           # x chunk width
NCH = S // CH      # 8
NQB = S // QB      # 8
NKT = S // KT      # 32
GRP = 2            # key tiles per exp group (2 PSUM banks per group)
NG = NKT // GRP    # 16 groups per query block

F32 = mybir.dt.float32
F32R = mybir.dt.float32r
F16 = mybir.dt.float16
Exp = mybir.ActivationFunctionType.Exp


def build_program():
    nc = bacc.Bacc("TRN2", target_bir_lowering=False, debug=False, num_devices=8)

    def di(name, shape, dt=F16):
        return nc.dram_tensor(name, shape, dt, kind="ExternalInput").ap()

    xT = di("xT", [HID, S])            # x[b].T
    wqk = di("wqk", [128, 256])        # [wqT_hi/8|wkT_hi ; wqT_lo/8|wkT_lo]
    wv2 = di("wv2", [128, 2 * DV])     # [wvT_hi | wvT_lo], col 64.. pad 0
    bias2 = di("bias2", [128, 2], F32) # col0 = [bq/8; bk]
    bvrow = di("bvrow", [1, DV])       # [bv | 1 | 0 0 0] as a row
    ones1 = di("ones1", [1, 128])      # ones row (stationary for bias mm)
    bsynT = di("bsynT", [D, S])        # (onehot @ syn_bias).T
    onehotT = di("onehotT", [D, S])
    woT = di("woT", [D, HID])          # wo[:, hslice].T
    outT = nc.dram_tensor("outT", [HID, S], F16, kind="ExternalOutput").ap()
    lT = nc.dram_tensor("lT", [1, S], F32, kind="ExternalOutput").ap()

    with tile.TileContext(nc) as tc:
        _body(tc, xT, wqk, wv2, bias2, bvrow, ones1, bsynT, onehotT, woT,
              outT, lT)
    nc.compile()
    return nc


def _body(tc, xT, wqk, wv2, bias2, bvrow, ones1, bsynT, onehotT, woT,
          outT, lT):
    nc = tc.nc
    mm = nc.tensor.matmul

    with (
        tc.tile_pool(name="const", bufs=1) as constp,
        tc.tile_pool(name="big", bufs=1) as bigp,
        tc.tile_pool(name="pt", bufs=12) as ptp,
        tc.tile_pool(name="ob", bufs=2) as obp,
    ):
        # ---- constants ----
        wqk_sb = constp.tile([128, 256], F16, name="wqk_sb", tag="wqk_sb")
        wv_sb = constp.tile([128, 2 * DV], F16, name="wv_sb", tag="wv_sb")
        b2_sb = constp.tile([128, 2], F32, name="b2_sb", tag="b2_sb")
        bv_sb = constp.tile([1, DV], F16, name="bv_sb", tag="bv_sb")
        ones_sb = constp.tile([1, 128], F16, name="ones_sb", tag="ones_sb")
        wo_sb = constp.tile([D, HID], F16, name="wo_sb", tag="wo_sb")
        scr = constp.tile([1, 1], F32, name="scr", tag="scr")

        # persistent activations (subregion deps make slices per-chunk)
        xc0 = [bigp.tile([128, CH], F16, name=f"xc0_{c}", tag=f"xc0_{c}")
               for c in range(NCH)]
        xc1 = [bigp.tile([128, CH], F16, name=f"xc1_{c}", tag=f"xc1_{c}")
               for c in range(NCH)]
        qTt = bigp.tile([128, S], F16, name="qTt", tag="qTt")  # 0:64 q/8, 64:128 bsynT
        kTt = bigp.tile([128, S], F16, name="kTt", tag="kTt")  # 0:64 k,   64:128 onehotT
        vb = bigp.tile([128, NKT * DV], F16, name="vb", tag="vb")  # v' key-major
        oall = bigp.tile([D, S], F16, name="oall", tag="oall")
        l_sb = bigp.tile([1, S], F32, name="l_sb", tag="l_sb")

        # ---- DMA issues. The scalar queue carries ZERO dma work: every
        # issue there (~0.6us each) serializes with the ACTIVATE stream,
        # which is the kernel's bottleneck engine. Sync (hwdge) takes the
        # weights + chunk-0 criticals + x chunks; GpSimd (swdge) takes the
        # remaining bias streams and, later, all mid-stream output DMAs.
        nc.sync.dma_start(wqk_sb[:], wqk[:])
        nc.sync.dma_start(b2_sb[:], bias2[:])
        nc.sync.dma_start(xc0[0][:], xT[0:128, 0:CH])
        nc.sync.dma_start(xc1[0][:], xT[128:256, 0:CH])
        nc.sync.dma_start(qTt[64:128, 0:CH], bsynT[:, 0:CH])
        nc.sync.dma_start(kTt[64:128, 0:CH], onehotT[:, 0:CH])
        nc.sync.dma_start(wv_sb[:], wv2[:])
        nc.sync.dma_start(bv_sb[:], bvrow[:])
        nc.sync.dma_start(ones_sb[:], ones1[:])
        for c in range(1, NCH):
            cs = slice(c * CH, (c + 1) * CH)
            nc.sync.dma_start(xc0[c][:], xT[0:128, cs])
            nc.sync.dma_start(xc1[c][:], xT[128:256, cs])
        for c in range(1, NCH):
            cs = slice(c * CH, (c + 1) * CH)
            nc.gpsimd.dma_start(qTt[64:128, cs], bsynT[:, cs])
            nc.gpsimd.dma_start(kTt[64:128, cs], onehotT[:, cs])
        nc.gpsimd.dma_start(wo_sb[:], woT[:])

        # warm the ACT exp table (~2.7us) while projections run
        nc.scalar.activation(scr[:], b2_sb[0:1, 0:1], Exp)

        with (
            tc.tile_pool(name="psB", bufs=2, space="PSUM") as psB,
            tc.tile_pool(name="psAcc", bufs=2, space="PSUM") as psAcc,
            tc.tile_pool(name="psX", bufs=2, space="PSUM") as psX,
        ):
            # ---- helpers ----
            def emit_qk(c, pool, on_act=False):
                cs = slice(c * CH, (c + 1) * CH)
                qkp = pool.tile([128, CH], F32, name="qkp",
                                tag="oacc" if pool is psAcc else "px")
                mm(qkp[:], wqk_sb[:, 0:128], xc0[c][:], start=True, stop=False)
                mm(qkp[:], wqk_sb[:, 128:256], xc1[c][:], start=False,
                   stop=True)
                nc.vector.tensor_scalar_add(qTt[0:D, cs], qkp[0:D, :],
                                            b2_sb[0:D, 0:1])
                nc.vector.tensor_scalar_add(kTt[0:D, cs], qkp[D:128, :],
                                            b2_sb[D:128, 0:1])

            def emit_v(c):
                # v' computed DIRECTLY key-major: out[key, d] with the x
                # chunk slice as stationary and the wv half as moving — only
                # 3x68 moving rows per key tile (vs 512-row projections plus
                # PE transposes). The [bv | 1] row rides in as a rank-1
                # matmul against a ones row, which also plants the
                # denominator ones column.
                vtr = psX.tile([128, 4 * DV], F32, name="vtr", tag="px")
                for m in range(4):
                    ks = slice(m * KT, (m + 1) * KT)
                    vs = slice(m * DV, (m + 1) * DV)
                    mm(vtr[:, vs], xc0[c][:, ks], wv_sb[:, 0:DV],
                       start=True, stop=False)
                    mm(vtr[:, vs], xc1[c][:, ks], wv_sb[:, DV:2 * DV],
                       start=False, stop=False)
                    mm(vtr[:, vs], ones_sb[:], bv_sb[:],
                       start=False, stop=True)
                nc.vector.tensor_copy(vb[:, 4 * c * DV:(4 * c + 4) * DV],
                                      vtr[:])

            oaccs = {}

            def proj_steps(qb):
                """Deferred output projection for query block qb; the oacc
                eviction happens immediately (DVE is idle), the PE matmuls
                run later in loose slots of the next block."""
                qsl = slice(qb * QB, (qb + 1) * QB)
                oacc = oaccs.pop(qb)
                nc.vector.tensor_copy(oall[:, qsl], oacc[0:D, :])
                nc.vector.tensor_copy(l_sb[:, qsl], oacc[LCOL:LCOL + 1, :])

                def s1():
                    pj = psX.tile([128, QB], F32, name="pj", tag="px")
                    ob = obp.tile([128, QB], F16, name="ob", tag="ob")
                    mm(pj[:], wo_sb[:, 0:128], oall[:, qsl],
                       start=True, stop=True)
                    nc.vector.tensor_copy(ob[:], pj[:])
                    nc.gpsimd.dma_start(outT[0:128, qsl], ob[:])

                def s2():
                    pj = psX.tile([128, QB], F32, name="pj", tag="px")
                    ob = obp.tile([128, QB], F16, name="ob", tag="ob")
                    mm(pj[:], wo_sb[:, 128:256], oall[:, qsl],
                       start=True, stop=True)
                    nc.vector.tensor_copy(ob[:], pj[:])
                    nc.gpsimd.dma_start(outT[128:256, qsl], ob[:])
                    nc.gpsimd.dma_start(lT[:, qsl], l_sb[:, qsl])

                return [s1, s2]

            def proj_last(qb):
                """Final block: same halves; casts split across Vector and
                GpSimd so they run in parallel, and the output DMAs go on
                the two hwdge queues (sync + the now-idle scalar) so no
                slow swdge drain sits at the very end."""
                qsl = slice(qb * QB, (qb + 1) * QB)
                oacc = oaccs.pop(qb)
                nc.vector.tensor_copy(oall[:, qsl], oacc[0:D, :])
                nc.vector.tensor_copy(l_sb[:, qsl], oacc[LCOL:LCOL + 1, :])
                nc.sync.dma_start(lT[:, qsl], l_sb[:, qsl])
                for half, ofs in ((0, 0), (1, 128)):
                    pj = psX.tile([128, QB], F32, name="pjl", tag="px")
                    ob = obp.tile([128, QB], F16, name="obl", tag="ob")
                    mm(pj[:], wo_sb[:, ofs:ofs + 128], oall[:, qsl],
                       start=True, stop=True)
                    if half == 0:
                        nc.vector.tensor_copy(ob[:], pj[:])
                    else:
                        # ACT is idle once the exp stream ends — use it for
                        # the second cast so the two halves run in parallel.
                        nc.scalar.activation(
                            ob[:], pj[:], mybir.ActivationFunctionType.Copy)
                    q0 = qb * QB
                    eng = (nc.sync, nc.scalar)
                    eng[half].dma_start(outT[ofs:ofs + 128, q0:q0 + 256],
                                        ob[:, 0:256])
                    eng[1 - half].dma_start(
                        outT[ofs:ofs + 128, q0 + 256:q0 + 512],
                        ob[:, 256:512])

            # PE p-state warmup: dummy matmuls on the first-arrived weights
            # bridge the gap until the x chunk-0 DMA lands (so qk0 doesn't
            # run at the cold 0.65GHz p-state). A few more after qk0 keep
            # the PE busy while the DVE bias-add produces qTt/kTt chunk 0,
            # preserving the p-state ramp into the score stream.
            warm = psX.tile([128, 256], F32, name="warm", tag="px")
            for _ in range(10):
                mm(warm[:], wqk_sb[:, 0:128], wqk_sb[:], start=True, stop=True)
            emit_qk(0, psX)
            for _ in range(4):
                mm(warm[:], wqk_sb[:, 0:128], wqk_sb[:], start=True, stop=True)

            # ---- injected work, placed just ahead of each deadline:
            # kTt chunk c feeds score groups 2c..2c+1 -> qk(c) at group
            # 2c-1; vb chunk c is first read by PV(2c) which drains at
            # group 2c+PVLAG -> emit_v(c) at 2c+2. PV is lagged by a deep
            # PVLAG=10 so qb0 carries no PV work at all -> the PE (which
            # also runs all the injected projections at the not-yet-ramped
            # p-state) can keep the score stream ahead of ACT.
            inject = {
                1: [lambda: emit_qk(1, psAcc)],
                2: [lambda: emit_v(0)],
                3: [lambda: emit_qk(2, psX)],
                4: [lambda: emit_v(1)],
                5: [lambda: emit_qk(3, psAcc)],
                6: [lambda: emit_v(2)],
                7: [lambda: emit_qk(4, psAcc)],
                8: [lambda: emit_v(3)],
                9: [lambda: emit_qk(5, psAcc)],
                10: [lambda: emit_v(4)],
                11: [lambda: emit_qk(6, psX)],
                12: [lambda: emit_v(5)],
                13: [lambda: emit_qk(7, psX)],
                14: [lambda: emit_v(6)],
            }
            inject_qb1 = {
                0: [lambda: emit_v(7)],
            }

            # ---- attention stream (PV lags scores by PVLAG groups) ----
            PVLAG = 10
            pv_queue = []
            pending_proj = []

            def emit_pv(qb, gi, p3):
                if gi == 0:
                    oaccs[qb] = psAcc.tile([DV, QB], F32, name="oacc",
                                           tag="oacc")
                oacc = oaccs[qb]
                for m in range(GRP):
                    j = GRP * gi + m
                    mm(oacc[:], vb[:, j * DV:(j + 1) * DV],
                       p3[:, m * QB:(m + 1) * QB],
                       start=(j == 0), stop=(j == NKT - 1))

            done_qb = {}

            def drain_one_pv():
                qb0_, gi0_, p30_ = pv_queue.pop(0)
                emit_pv(qb0_, gi0_, p30_)
                if gi0_ == NG - 1:
                    done_qb[qb0_] = True

            for qb in range(NQB):
                qsl = slice(qb * QB, (qb + 1) * QB)
                for gi in range(NG):
                    if qb == 0:
                        for thunk in inject.get(gi, ()):
                            thunk()
                    elif qb == 1:
                        for thunk in inject_qb1.get(gi, ()):
                            thunk()
                    s3 = psB.tile([128, GRP * QB], F32, name="s3", tag="s3")
                    for m in range(GRP):
                        j = GRP * gi + m
                        jl = slice(j * KT, (j + 1) * KT)
                        mm(s3[:, m * QB:(m + 1) * QB], kTt[:, jl], qTt[:, qsl],
                           start=True, stop=True)
                    p3 = ptp.tile([128, GRP * QB], F16, name="p3", tag="p3")
                    nc.scalar.activation(p3[:], s3[:], Exp)
                    pv_queue.append((qb, gi, p3))
                    # Last two blocks: taper the PV lag (max 2 drains per
                    # group) so the post-stream tail is one PV group instead
                    # of PVLAG+1.
                    if qb < NQB - 2:
                        lag = PVLAG
                    elif qb == NQB - 2:
                        lag = 5
                    else:
                        lag = 1
                    drains = 0
                    while len(pv_queue) > lag and drains < 2:
                        drain_one_pv()
                        drains += 1
                    # previous block done accumulating? evict + start proj
                    # (extend, never replace: with the taper, done_qb can
                    # fire while the previous block's s2 is still pending)
                    if done_qb.pop(qb - 1, None):
                        pending_proj.extend(proj_steps(qb - 1))
                    if pending_proj and (gi % 6 == 5 or
                                         (qb == NQB - 1 and gi % 3 == 2)):
                        pending_proj.pop(0)()
            while pv_queue:
                drain_one_pv()
            for step in pending_proj:
                step()
            proj_last(NQB - 1)


_NC_CACHE = {}


def _get_program():
    if "nc" not in _NC_CACHE:
        _NC_CACHE["nc"] = build_program()
    return _NC_CACHE["nc"]


def make_in_maps(x, codons, syn_bias, wq, bq, wk, bk, wv, bv, wo):
    in_maps = []
    for core in range(8):
        b, h = divmod(core, NH)
        hsl = slice(h * D, (h + 1) * D)
        cod = codons[b]
        onehotT = np.zeros((D, S), np.float16)
        onehotT[cod, np.arange(S)] = 1.0
        # [wqT/8 | wkT] packed as [hi-half ; lo-half] -> [128, 256]
        wqk_full = np.concatenate([wq[hsl, :].T / 8.0, wk[hsl, :].T], axis=1)
        wqk = np.concatenate([wqk_full[0:128], wqk_full[128:256]], axis=1)
        wvp = np.concatenate(
            [wv[hsl, :].T, np.zeros((HID, DV - D), np.float32)], axis=1)
        wv2 = np.concatenate([wvp[0:128], wvp[128:256]], axis=1)
        bias2 = np.zeros((128, 2), np.float32)
        bias2[:, 0] = np.concatenate([bq[hsl] / 8.0, bk[hsl]])
        bvrow = np.zeros((1, DV), np.float32)
        bvrow[0, 0:D] = bv[hsl]
        bvrow[0, LCOL] = 1.0
        in_maps.append({
            "xT": x[b].T.astype(np.float16),
            "wqk": wqk.astype(np.float16),
            "wv2": wv2.astype(np.float16),
            "bias2": bias2,
            "bvrow": bvrow.astype(np.float16),
            "ones1": np.ones((1, 128), np.float16),
            "bsynT": np.ascontiguousarray(syn_bias.T[:, cod]).astype(np.float16),
            "onehotT": onehotT,
            "woT": wo[:, hsl].T.astype(np.float16),
        })
    return in_maps


def kernel_run(inputs, trace=False):
    x = np.asarray(inputs["x"], np.float32)
    codons = np.asarray(inputs["codons"]).astype(np.int64)
    syn_bias = np.asarray(inputs["syn_bias"], np.float32)
    wq = np.asarray(inputs["wq"], np.float32)
    bq = np.asarray(inputs["bq"], np.float32)
    wk = np.asarray(inputs["wk"], np.float32)
    bk = np.asarray(inputs["bk"], np.float32)
    wv = np.asarray(inputs["wv"], np.float32)
    bv = np.asarray(inputs["bv"], np.float32)
    wo = np.asarray(inputs["wo"], np.float32)
    bo = np.asarray(inputs["bo"], np.float32)

    nc = _get_program()
    in_maps = make_in_maps(x, codons, syn_bias, wq, bq, wk, bk, wv, bv, wo)
    res = run_bass_kernel_spmd(nc, in_maps, core_ids=list(range(8)), trace=trace)

    out = np.empty((B, S, HID), np.float32)
    for b in range(B):
        acc = None
        for h in range(NH):
            r = res.results[NH * b + h]
            part = r["outT"].astype(np.float32) / r["lT"]   # normalize per head
            acc = part if acc is None else acc + part
        out[b] = acc.T + bo
    return out, res


def kernel(**inputs):
    out, _ = kernel_run(inputs, trace=False)
    return out



# revision 20
# speedup vs baseline: 1.0308x; 1.0012x over previous
"""CodonAttention Trainium2 kernel (fp16 stream, issue-lean pipeline).

Math (per batch b, head h):
  q = x @ wq.T + bq ; k = x @ wk.T + bk ; v = x @ wv.T + bv   (head slices)
  scores = q k^T / 8 + syn_bias[codons_i, codons_j]
  out    = softmax(scores) @ v ;  final = concat_heads(out) @ wo.T + bo

Key algebraic trick: the pairwise codon bias factors through one-hots,
  pair_bias = onehot @ syn_bias @ onehot.T
so augmenting q' = [(q+bq)/8 | onehot @ syn_bias] and k' = [k | onehot] gives
  scores = q' @ k'.T        (effective head dim 128 — exactly one partition)
Softmax runs without max-subtraction (|scores| <= ~4.3, exp safe in fp32) and
the row-sum l is obtained with a ones-column in v: [O | l] = P @ [v | 1].

Sharding: 8 cores = (batch b in {0,1}) x (head h in {0..3}). Each core runs
the full attention for its (b, h), producing the UNNORMALIZED partial
projection outT = (wo_h @ O_h.T) (256, 4096) plus denominators lT (1, 4096);
the host divides, sums the 4 head partials per batch, transposes, adds bo.

Profile-driven design (trace facts from this hardware):
- Phase B is ACT-bound: exp runs 1 col/cycle @1.2GHz regardless of dtype,
  ~1.0us per [128,1024] group; the PE streams 512-row fp16 matmuls at
  ~0.42 ns/row so 4 matmuls/group (~0.87us) fit under the exp.
- Every dma_start costs ~0.6-1us of *issue* time on its queue, so DMA
  issues are spread: Sync + Scalar queues carry x/weights (hwdge),
  GpSimd carries the bias streams and all output DMAs (swdge, idle
  engine). Queue order puts chunk 0 first so compute starts ~3us in.
- Engines execute their queue in order, so late-chunk work must not sit
  in front of the attention stream: q/k projections and v transposes
  for chunks 3..7 are injected INTO the qb-0 attention stream right
  before the groups that consume them.
- The per-block output projection is deferred two groups into the next
  query block so its oacc->oall->PE chain never stalls the score
  pipeline (it runs in loose slots, PSUM bank shared with the qk
  projection pool).
- PSUM budget (8 banks): scores double-buffer 2x2 + oacc 2 + v-flip 1 +
  qkproj/outproj shared 1.
"""

import numpy as np

import concourse.mybir as mybir
import concourse.tile as tile
from concourse import bacc
from concourse.bass_utils import run_bass_kernel_spmd


def _ensure_axon_ntff_hook():
    """This image's antenv package lacks axon_hooks, so
    run_bass_kernel_spmd(trace=True) (or BASS_TRACE=1) would die on the
    import. Register a compatible module backed by the libaxon_pjrt C ABI
    so tracing works if a caller requests it."""
    import sys
    try:
        import antenv.axon_hooks  # noqa: F401
        return
    except ImportError:
        pass
    import contextlib
    import ctypes
    import types
    try:
        lib = ctypes.CDLL("/opt/axon/libaxon_pjrt.so")
        has = hasattr(lib, "axon_start_nrt_profile")
    except OSError:
        has = False
    if has:
        lib.axon_start_nrt_profile.argtypes = [ctypes.POINTER(ctypes.c_int64),
                                               ctypes.c_size_t]
        lib.axon_start_nrt_profile.restype = ctypes.c_int64
        lib.axon_stop_nrt_profile.argtypes = [ctypes.c_char_p]
        lib.axon_stop_nrt_profile.restype = ctypes.c_int64

        @contextlib.contextmanager
        def _hook(output_dir, device_ids):
            import jax
            jax.devices()
            if device_ids:
                ids = (ctypes.c_int64 * len(device_ids))(*device_ids)
                rc = lib.axon_start_nrt_profile(ids, len(device_ids))
            else:
                rc = lib.axon_start_nrt_profile(None, 0)
            if rc != 0:
                raise RuntimeError(f"axon_start_nrt_profile rc={rc}")
            try:
                yield
            finally:
                lib.axon_stop_nrt_profile(str(output_dir).encode())
    else:
        _hook = None

    mod = types.ModuleType("antenv.axon_hooks")
    _state = {"hook": _hook}
    mod.get_axon_ntff_profile_hook = lambda: _state["hook"]
    mod.set_axon_ntff_profile_hook = lambda h: _state.__setitem__("hook", h)
    sys.modules["antenv.axon_hooks"] = mod


_ensure_axon_ntff_hook()

B, S, HID, NH, D = 2, 4096, 256, 4, 64
DV = D + 4         # v cols + ones column + 3 pad
LCOL = D           # index of the ones column inside a v tile
QB = 512           # query block (free dim of score matmuls)
KT = 128           # key tile (partition dim of transposed scores)
CH = 512           # x chunk width
NCH = S // CH      # 8
NQB = S // QB      # 8
NKT = S // KT      # 32
GRP = 2            # key tiles per exp group (2 PSUM banks per group)
NG = NKT // GRP    # 16 groups per query block

F32 = mybir.dt.float32
F32R = mybir.dt.float32r
F16 = mybir.dt.float16
Exp = mybir.ActivationFunctionType.Exp


def build_program():
    nc = bacc.Bacc("TRN2", target_bir_lowering=False, debug=False, num_devices=8)

    def di(name, shape, dt=F16):
        return nc.dram_tensor(name, shape, dt, kind="ExternalInput").ap()

    xT = di("xT", [HID, S])            # x[b].T
    wqk = di("wqk", [128, 256])        # [wqT_hi/8|wkT_hi ; wqT_lo/8|wkT_lo]
    wv2 = di("wv2", [128, 2 * DV])     # [wvT_hi | wvT_lo], col 64.. pad 0
    bias2 = di("bias2", [128, 2], F32) # col0 = [bq/8; bk]
    bvrow = di("bvrow", [1, DV])       # [bv | 1 | 0 0 0] as a row
    ones1 = di("ones1", [1, 128])      # ones row (stationary for bias mm)
    bsynT = di("bsynT", [D, S])        # (onehot @ syn_bias).T
    onehotT = di("onehotT", [D, S])
    woT = di("woT", [D, HID])          # wo[:, hslice].T
    outT = nc.dram_tensor("outT", [HID, S], F16, kind="ExternalOutput").ap()
    lT = nc.dram_tensor("lT", [1, S], F32, kind="ExternalOutput").ap()

    with tile.TileContext(nc) as tc:
        _body(tc, xT, wqk, wv2, bias2, bvrow, ones1, bsynT, onehotT, woT,
              outT, lT)
    nc.compile()
    return nc


def _body(tc, xT, wqk, wv2, bias2, bvrow, ones1, bsynT, onehotT, woT,
          outT, lT):
    nc = tc.nc
    mm = nc.tensor.matmul

    with (
        tc.tile_pool(name="const", bufs=1) as constp,
        tc.tile_pool(name="big", bufs=1) as bigp,
        tc.tile_pool(name="pt", bufs=12) as ptp,
        tc.tile_pool(name="ob", bufs=2) as obp,
    ):
        # ---- constants ----
        wqk_sb = constp.tile([128, 256], F16, name="wqk_sb", tag="wqk_sb")
        wv_sb = constp.tile([128, 2 * DV], F16, name="wv_sb", tag="wv_sb")
        b2_sb = constp.tile([128, 2], F32, name="b2_sb", tag="b2_sb")
        bv_sb = constp.tile([1, DV], F16, name="bv_sb", tag="bv_sb")
        ones_sb = constp.tile([1, 128], F16, name="ones_sb", tag="ones_sb")
        wo_sb = constp.tile([D, HID], F16, name="wo_sb", tag="wo_sb")
        scr = constp.tile([1, 1], F32, name="scr", tag="scr")

        # persistent activations (subregion deps make slices per-chunk)
        xc0 = [bigp.tile([128, CH], F16, name=f"xc0_{c}", tag=f"xc0_{c}")
               for c in range(NCH)]
        xc1 = [bigp.tile([128, CH], F16, name=f"xc1_{c}", tag=f"xc1_{c}")
               for c in range(NCH)]
        qTt = bigp.tile([128, S], F16, name="qTt", tag="qTt")  # 0:64 q/8, 64:128 bsynT
        kTt = bigp.tile([128, S], F16, name="kTt", tag="kTt")  # 0:64 k,   64:128 onehotT
        vb = bigp.tile([128, NKT * DV], F16, name="vb", tag="vb")  # v' key-major
        oall = bigp.tile([D, S], F16, name="oall", tag="oall")
        l_sb = bigp.tile([1, S], F32, name="l_sb", tag="l_sb")

        # ---- DMA issues. Each hwdge issue costs its queue ~0.65us, and
        # chunk DELIVERY paces the PE injects (a late chunk gaps the PE,
        # which drops its p-state to 1.2GHz for the next 3us and starves
        # the exp stream). So: the two hwdge queues split the chunk-0/1
        # criticals in parallel (scalar is free until the first exp, ~13us);
        # gpsimd (swdge) streams the per-chunk trios for chunks 2..7 in
        # consumption order, earliest first. The scalar queue carries
        # nothing after the exp stream starts.
        nc.sync.dma_start(wqk_sb[:], wqk[:])
        nc.sync.dma_start(xc0[0][:], xT[0:128, 0:CH])
        nc.sync.dma_start(qTt[64:128, 0:CH], bsynT[:, 0:CH])
        nc.sync.dma_start(b2_sb[:], bias2[:])
        nc.sync.dma_start(wv_sb[:], wv2[:])
        nc.sync.dma_start(bv_sb[:], bvrow[:])
        nc.sync.dma_start(ones_sb[:], ones1[:])
        for c in range(1, NCH):
            nc.sync.dma_start(xc0[c][:], xT[0:128, c * CH:(c + 1) * CH])
        nc.scalar.dma_start(xc1[0][:], xT[128:256, 0:CH])
        nc.scalar.dma_start(kTt[64:128, 0:CH], onehotT[:, 0:CH])
        nc.scalar.dma_start(xc1[1][:], xT[128:256, CH:2 * CH])
        nc.scalar.dma_start(qTt[64:128, CH:2 * CH], bsynT[:, CH:2 * CH])
        nc.scalar.dma_start(kTt[64:128, CH:2 * CH], onehotT[:, CH:2 * CH])
        for c in range(2, NCH):
            cs = slice(c * CH, (c + 1) * CH)
            nc.gpsimd.dma_start(xc1[c][:], xT[128:256, cs])
            nc.gpsimd.dma_start(qTt[64:128, cs], bsynT[:, cs])
            nc.gpsimd.dma_start(kTt[64:128, cs], onehotT[:, cs])
        nc.gpsimd.dma_start(wo_sb[:], woT[:])

        # warm the ACT exp table (~2.7us) while projections run
        nc.scalar.activation(scr[:], b2_sb[0:1, 0:1], Exp)

        with (
            tc.tile_pool(name="psB", bufs=2, space="PSUM") as psB,
            tc.tile_pool(name="psAcc", bufs=2, space="PSUM") as psAcc,
            tc.tile_pool(name="psX", bufs=2, space="PSUM") as psX,
        ):
            # ---- helpers ----
            def emit_qk(c, pool, on_act=False):
                cs = slice(c * CH, (c + 1) * CH)
                qkp = pool.tile([128, CH], F32, name="qkp",
                                tag="oacc" if pool is psAcc else "px")
                mm(qkp[:], wqk_sb[:, 0:128], xc0[c][:], start=True, stop=False)
                mm(qkp[:], wqk_sb[:, 128:256], xc1[c][:], start=False,
                   stop=True)
                # kT eviction FIRST: during qb0 the next score group waits
                # only on kTt (qTt chunk c isn't read until qb c), so the
                # qT eviction stays off the critical chain.
                nc.vector.tensor_scalar_add(kTt[0:D, cs], qkp[D:128, :],
                                            b2_sb[D:128, 0:1])
                nc.vector.tensor_scalar_add(qTt[0:D, cs], qkp[0:D, :],
                                            b2_sb[0:D, 0:1])

            def emit_v(c):
                # v' computed DIRECTLY key-major: out[key, d] with the x
                # chunk slice as stationary and the wv half as moving — only
                # 3x68 moving rows per key tile (vs 512-row projections plus
                # PE transposes). The [bv | 1] row rides in as a rank-1
                # matmul against a ones row, which also plants the
                # denominator ones column.
                vtr = psX.tile([128, 4 * DV], F32, name="vtr", tag="px")
                for m in range(4):
                    ks = slice(m * KT, (m + 1) * KT)
                    vs = slice(m * DV, (m + 1) * DV)
                    mm(vtr[:, vs], xc0[c][:, ks], wv_sb[:, 0:DV],
                       start=True, stop=False)
                    mm(vtr[:, vs], xc1[c][:, ks], wv_sb[:, DV:2 * DV],
                       start=False, stop=False)
                    mm(vtr[:, vs], ones_sb[:], bv_sb[:],
                       start=False, stop=True)
                nc.vector.tensor_copy(vb[:, 4 * c * DV:(4 * c + 4) * DV],
                                      vtr[:])

            oaccs = {}

            def proj_steps(qb):
                """Deferred output projection for query block qb; the oacc
                eviction happens immediately (DVE is idle), the PE matmuls
                run later in loose slots of the next block."""
                qsl = slice(qb * QB, (qb + 1) * QB)
                oacc = oaccs.pop(qb)
                nc.vector.tensor_copy(oall[:, qsl], oacc[0:D, :])
                nc.vector.tensor_copy(l_sb[:, qsl], oacc[LCOL:LCOL + 1, :])

                def s1():
                    pj = psX.tile([128, QB], F32, name="pj", tag="px")
                    ob = obp.tile([128, QB], F16, name="ob", tag="ob")
                    mm(pj[:], wo_sb[:, 0:128], oall[:, qsl],
                       start=True, stop=True)
                    nc.vector.tensor_copy(ob[:], pj[:])
                    nc.gpsimd.dma_start(outT[0:128, qsl], ob[:])

                def s2():
                    pj = psX.tile([128, QB], F32, name="pj", tag="px")
                    ob = obp.tile([128, QB], F16, name="ob", tag="ob")
                    mm(pj[:], wo_sb[:, 128:256], oall[:, qsl],
                       start=True, stop=True)
                    nc.vector.tensor_copy(ob[:], pj[:])
                    nc.gpsimd.dma_start(outT[128:256, qsl], ob[:])
                    nc.gpsimd.dma_start(lT[:, qsl], l_sb[:, qsl])

                return [s1, s2]

            def proj_last(qb):
                """Final block: same halves; casts split across Vector and
                GpSimd so they run in parallel, and the output DMAs go on
                the two hwdge queues (sync + the now-idle scalar) so no
                slow swdge drain sits at the very end."""
                qsl = slice(qb * QB, (qb + 1) * QB)
                oacc = oaccs.pop(qb)
                nc.vector.tensor_copy(oall[:, qsl], oacc[0:D, :])
                nc.vector.tensor_copy(l_sb[:, qsl], oacc[LCOL:LCOL + 1, :])
                nc.sync.dma_start(lT[:, qsl], l_sb[:, qsl])
                for half, ofs in ((0, 0), (1, 128)):
                    pj = psX.tile([128, QB], F32, name="pjl", tag="px")
                    ob = obp.tile([128, QB], F16, name="obl", tag="ob")
                    mm(pj[:], wo_sb[:, ofs:ofs + 128], oall[:, qsl],
                       start=True, stop=True)
                    if half == 0:
                        nc.vector.tensor_copy(ob[:], pj[:])
                    else:
                        # ACT is idle once the exp stream ends — use it for
                        # the second cast so the two halves run in parallel.
                        nc.scalar.activation(
                            ob[:], pj[:], mybir.ActivationFunctionType.Copy)
                    q0 = qb * QB
                    eng = (nc.sync, nc.scalar)
                    eng[half].dma_start(outT[ofs:ofs + 128, q0:q0 + 256],
                                        ob[:, 0:256])
                    eng[1 - half].dma_start(
                        outT[ofs:ofs + 128, q0 + 256:q0 + 512],
                        ob[:, 256:512])

            # PE p-state warmup: dummy matmuls on the first-arrived weights
            # bridge the gap until the x chunk-0 DMA lands (so qk0 doesn't
            # run at the cold 0.65GHz p-state). A few more after qk0 keep
            # the PE busy while the DVE bias-add produces qTt/kTt chunk 0,
            # preserving the p-state ramp into the score stream.
            warm = psX.tile([128, 256], F32, name="warm", tag="px")
            for _ in range(10):
                mm(warm[:], wqk_sb[:, 0:128], wqk_sb[:], start=True, stop=True)
            emit_qk(0, psX)
            for _ in range(4):
                mm(warm[:], wqk_sb[:, 0:128], wqk_sb[:], start=True, stop=True)

            # ---- injected work, placed just ahead of each deadline:
            # kTt chunk c feeds score groups 2c..2c+1 -> qk(c) at group
            # 2c-1; vb chunk c is first read by PV(2c) which drains at
            # group 2c+PVLAG -> emit_v(c) at 2c+2. PV is lagged by a deep
            # PVLAG=10 so qb0 carries no PV work at all -> the PE (which
            # also runs all the injected projections at the not-yet-ramped
            # p-state) can keep the score stream ahead of ACT.
            inject = {
                1: [lambda: emit_qk(1, psAcc)],
                2: [lambda: emit_v(0)],
                3: [lambda: emit_qk(2, psX)],
                4: [lambda: emit_v(1)],
                5: [lambda: emit_qk(3, psAcc)],
                6: [lambda: emit_v(2)],
                7: [lambda: emit_qk(4, psAcc)],
                8: [lambda: emit_v(3)],
                9: [lambda: emit_qk(5, psAcc)],
                10: [lambda: emit_v(4)],
                11: [lambda: emit_qk(6, psX)],
                12: [lambda: emit_v(5)],
                13: [lambda: emit_qk(7, psX)],
                14: [lambda: emit_v(6)],
            }
            inject_qb1 = {
                0: [lambda: emit_v(7)],
            }

            # ---- attention stream (PV lags scores by PVLAG groups) ----
            PVLAG = 10
            pv_queue = []
            pending_proj = []

            def emit_pv(qb, gi, p3):
                if gi == 0:
                    oaccs[qb] = psAcc.tile([DV, QB], F32, name="oacc",
                                           tag="oacc")
                oacc = oaccs[qb]
                for m in range(GRP):
                    j = GRP * gi + m
                    mm(oacc[:], vb[:, j * DV:(j + 1) * DV],
                       p3[:, m * QB:(m + 1) * QB],
                       start=(j == 0), stop=(j == NKT - 1))

            done_qb = {}

            def drain_one_pv():
                qb0_, gi0_, p30_ = pv_queue.pop(0)
                emit_pv(qb0_, gi0_, p30_)
                if gi0_ == NG - 1:
                    done_qb[qb0_] = True

            for qb in range(NQB):
                qsl = slice(qb * QB, (qb + 1) * QB)
                for gi in range(NG):
                    if qb == 0:
                        for thunk in inject.get(gi, ()):
                            thunk()
                    elif qb == 1:
                        for thunk in inject_qb1.get(gi, ()):
                            thunk()
                    s3 = psB.tile([128, GRP * QB], F32, name="s3", tag="s3")
                    for m in range(GRP):
                        j = GRP * gi + m
                        jl = slice(j * KT, (j + 1) * KT)
                        mm(s3[:, m * QB:(m + 1) * QB], kTt[:, jl], qTt[:, qsl],
                           start=True, stop=True)
                    p3 = ptp.tile([128, GRP * QB], F16, name="p3", tag="p3")
                    nc.scalar.activation(p3[:], s3[:], Exp)
                    pv_queue.append((qb, gi, p3))
                    # Last three blocks: taper the PV lag gradually (the
                    # extra drains sit mid-block, away from the boundary
                    # where the PE p-state is still recovering) so the
                    # post-stream tail is one PV group instead of PVLAG+1.
                    if qb < NQB - 3:
                        lag = PVLAG
                    elif qb == NQB - 3:
                        lag = PVLAG - min(2, max(0, gi - 8))
                    elif qb == NQB - 2:
                        lag = 8 - min(3, max(0, gi - 5))
                    else:
                        lag = 5 - min(4, max(0, gi - 4))
                    drains = 0
                    while len(pv_queue) > lag and drains < 2:
                        drain_one_pv()
                        drains += 1
                    # previous block done accumulating? evict + start proj
                    # (extend, never replace: with the taper, done_qb can
                    # fire while the previous block's s2 is still pending)
                    if done_qb.pop(qb - 1, None):
                        pending_proj.extend(proj_steps(qb - 1))
                    if pending_proj and (gi % 6 == 5 or
                                         (qb == NQB - 1 and gi % 3 == 2)):
                        pending_proj.pop(0)()
            while pv_queue:
                drain_one_pv()
            for step in pending_proj:
                step()
            proj_last(NQB - 1)


_NC_CACHE = {}


def _get_program():
    if "nc" not in _NC_CACHE:
        _NC_CACHE["nc"] = build_program()
    return _NC_CACHE["nc"]


def make_in_maps(x, codons, syn_bias, wq, bq, wk, bk, wv, bv, wo):
    in_maps = []
    for core in range(8):
        b, h = divmod(core, NH)
        hsl = slice(h * D, (h + 1) * D)
        cod = codons[b]
        onehotT = np.zeros((D, S), np.float16)
        onehotT[cod, np.arange(S)] = 1.0
        # [wqT/8 | wkT] packed as [hi-half ; lo-half] -> [128, 256]
        wqk_full = np.concatenate([wq[hsl, :].T / 8.0, wk[hsl, :].T], axis=1)
        wqk = np.concatenate([wqk_full[0:128], wqk_full[128:256]], axis=1)
        wvp = np.concatenate(
            [wv[hsl, :].T, np.zeros((HID, DV - D), np.float32)], axis=1)
        wv2 = np.concatenate([wvp[0:128], wvp[128:256]], axis=1)
        bias2 = np.zeros((128, 2), np.float32)
        bias2[:, 0] = np.concatenate([bq[hsl] / 8.0, bk[hsl]])
        bvrow = np.zeros((1, DV), np.float32)
        bvrow[0, 0:D] = bv[hsl]
        bvrow[0, LCOL] = 1.0
        in_maps.append({
            "xT": x[b].T.astype(np.float16),
            "wqk": wqk.astype(np.float16),
            "wv2": wv2.astype(np.float16),
            "bias2": bias2,
            "bvrow": bvrow.astype(np.float16),
            "ones1": np.ones((1, 128), np.float16),
            "bsynT": np.ascontiguousarray(syn_bias.T[:, cod]).astype(np.float16),
            "onehotT": onehotT,
            "woT": wo[:, hsl].T.astype(np.float16),
        })
    return in_maps


def kernel_run(inputs, trace=False):
    x = np.asarray(inputs["x"], np.float32)
    codons = np.asarray(inputs["codons"]).astype(np.int64)
    syn_bias = np.asarray(inputs["syn_bias"], np.float32)
    wq = np.asarray(inputs["wq"], np.float32)
    bq = np.asarray(inputs["bq"], np.float32)
    wk = np.asarray(inputs["wk"], np.float32)
    bk = np.asarray(inputs["bk"], np.float32)
    wv = np.asarray(inputs["wv"], np.float32)
    bv = np.asarray(inputs["bv"], np.float32)
    wo = np.asarray(inputs["wo"], np.float32)
    bo = np.asarray(inputs["bo"], np.float32)

    nc = _get_program()
    in_maps = make_in_maps(x, codons, syn_bias, wq, bq, wk, bk, wv, bv, wo)
    res = run_bass_kernel_spmd(nc, in_maps, core_ids=list(range(8)), trace=trace)

    out = np.empty((B, S, HID), np.float32)
    for b in range(B):
        acc = None
        for h in range(NH):
            r = res.results[NH * b + h]
            part = r["outT"].astype(np.float32) / r["lT"]   # normalize per head
            acc = part if acc is None else acc + part
        out[b] = acc.T + bo
    return out, res


def kernel(**inputs):
    out, _ = kernel_run(inputs, trace=False)
    return out



# revision 22
# speedup vs baseline: 1.0368x; 1.0058x over previous
"""CodonAttention Trainium2 kernel (fp16 stream, issue-lean pipeline).

Math (per batch b, head h):
  q = x @ wq.T + bq ; k = x @ wk.T + bk ; v = x @ wv.T + bv   (head slices)
  scores = q k^T / 8 + syn_bias[codons_i, codons_j]
  out    = softmax(scores) @ v ;  final = concat_heads(out) @ wo.T + bo

Key algebraic trick: the pairwise codon bias factors through one-hots,
  pair_bias = onehot @ syn_bias @ onehot.T
so augmenting q' = [(q+bq)/8 | onehot @ syn_bias] and k' = [k | onehot] gives
  scores = q' @ k'.T        (effective head dim 128 — exactly one partition)
Softmax runs without max-subtraction (|scores| <= ~4.3, exp safe in fp32) and
the row-sum l is obtained with a ones-column in v: [O | l] = P @ [v | 1].

Sharding: 8 cores = (batch b in {0,1}) x (head h in {0..3}). Each core runs
the full attention for its (b, h), producing the UNNORMALIZED partial
projection outT = (wo_h @ O_h.T) (256, 4096) plus denominators lT (1, 4096);
the host divides, sums the 4 head partials per batch, transposes, adds bo.

Profile-driven design (trace facts from this hardware):
- Phase B is ACT-bound: exp runs 1 col/cycle @1.2GHz regardless of dtype,
  ~1.0us per [128,1024] group; the PE streams 512-row fp16 matmuls at
  ~0.42 ns/row so 4 matmuls/group (~0.87us) fit under the exp.
- Every dma_start costs ~0.6-1us of *issue* time on its queue, so DMA
  issues are spread: Sync + Scalar queues carry x/weights (hwdge),
  GpSimd carries the bias streams and all output DMAs (swdge, idle
  engine). Queue order puts chunk 0 first so compute starts ~3us in.
- Engines execute their queue in order, so late-chunk work must not sit
  in front of the attention stream: q/k projections and v transposes
  for chunks 3..7 are injected INTO the qb-0 attention stream right
  before the groups that consume them.
- The per-block output projection is deferred two groups into the next
  query block so its oacc->oall->PE chain never stalls the score
  pipeline (it runs in loose slots, PSUM bank shared with the qk
  projection pool).
- PSUM budget (8 banks): scores double-buffer 2x2 + oacc 2 + v-flip 1 +
  qkproj/outproj shared 1.
"""

import numpy as np

import concourse.mybir as mybir
import concourse.tile as tile
from concourse import bacc
from concourse.bass_utils import run_bass_kernel_spmd


def _ensure_axon_ntff_hook():
    """This image's antenv package lacks axon_hooks, so
    run_bass_kernel_spmd(trace=True) (or BASS_TRACE=1) would die on the
    import. Register a compatible module backed by the libaxon_pjrt C ABI
    so tracing works if a caller requests it."""
    import sys
    try:
        import antenv.axon_hooks  # noqa: F401
        return
    except ImportError:
        pass
    import contextlib
    import ctypes
    import types
    try:
        lib = ctypes.CDLL("/opt/axon/libaxon_pjrt.so")
        has = hasattr(lib, "axon_start_nrt_profile")
    except OSError:
        has = False
    if has:
        lib.axon_start_nrt_profile.argtypes = [ctypes.POINTER(ctypes.c_int64),
                                               ctypes.c_size_t]
        lib.axon_start_nrt_profile.restype = ctypes.c_int64
        lib.axon_stop_nrt_profile.argtypes = [ctypes.c_char_p]
        lib.axon_stop_nrt_profile.restype = ctypes.c_int64

        @contextlib.contextmanager
        def _hook(output_dir, device_ids):
            import jax
            jax.devices()
            if device_ids:
                ids = (ctypes.c_int64 * len(device_ids))(*device_ids)
                rc = lib.axon_start_nrt_profile(ids, len(device_ids))
            else:
                rc = lib.axon_start_nrt_profile(None, 0)
            if rc != 0:
                raise RuntimeError(f"axon_start_nrt_profile rc={rc}")
            try:
                yield
            finally:
                lib.axon_stop_nrt_profile(str(output_dir).encode())
    else:
        _hook = None

    mod = types.ModuleType("antenv.axon_hooks")
    _state = {"hook": _hook}
    mod.get_axon_ntff_profile_hook = lambda: _state["hook"]
    mod.set_axon_ntff_profile_hook = lambda h: _state.__setitem__("hook", h)
    sys.modules["antenv.axon_hooks"] = mod


_ensure_axon_ntff_hook()

B, S, HID, NH, D = 2, 4096, 256, 4, 64
DV = D + 4         # v cols + ones column + 3 pad
LCOL = D           # index of the ones column inside a v tile
QB = 512           # query block (free dim of score matmuls)
KT = 128           # key tile (partition dim of transposed scores)
CH = 512           # x chunk width
NCH = S // CH      # 8
NQB = S // QB      # 8
NKT = S // KT      # 32
GRP = 2            # key tiles per exp group (2 PSUM banks per group)
NG = NKT // GRP    # 16 groups per query block

F32 = mybir.dt.float32
F32R = mybir.dt.float32r
F16 = mybir.dt.float16
Exp = mybir.ActivationFunctionType.Exp


def build_program():
    nc = bacc.Bacc("TRN2", target_bir_lowering=False, debug=False, num_devices=8)

    def di(name, shape, dt=F16):
        return nc.dram_tensor(name, shape, dt, kind="ExternalInput").ap()

    xT = di("xT", [HID, S])            # x[b].T
    wqk = di("wqk", [128, 256])        # [wqT_hi/8|wkT_hi ; wqT_lo/8|wkT_lo]
    wv2 = di("wv2", [128, 2 * DV])     # [wvT_hi | wvT_lo], col 64.. pad 0
    bias2 = di("bias2", [128, 2], F32) # col0 = [bq/8; bk]
    bvrow = di("bvrow", [1, DV])       # [bv | 1 | 0 0 0] as a row
    ones1 = di("ones1", [1, 128])      # ones row (stationary for bias mm)
    bsynT = di("bsynT", [D, S])        # (onehot @ syn_bias).T
    onehotT = di("onehotT", [D, S])
    woT = di("woT", [D, HID])          # wo[:, hslice].T
    outT = nc.dram_tensor("outT", [HID, S], F16, kind="ExternalOutput").ap()
    lT = nc.dram_tensor("lT", [1, S], F32, kind="ExternalOutput").ap()

    with tile.TileContext(nc) as tc:
        _body(tc, xT, wqk, wv2, bias2, bvrow, ones1, bsynT, onehotT, woT,
              outT, lT)
    nc.compile()
    return nc


def _body(tc, xT, wqk, wv2, bias2, bvrow, ones1, bsynT, onehotT, woT,
          outT, lT):
    nc = tc.nc
    mm = nc.tensor.matmul

    with (
        tc.tile_pool(name="const", bufs=1) as constp,
        tc.tile_pool(name="big", bufs=1) as bigp,
        tc.tile_pool(name="pt", bufs=12) as ptp,
        tc.tile_pool(name="ob", bufs=2) as obp,
    ):
        # ---- constants ----
        wqk_sb = constp.tile([128, 256], F16, name="wqk_sb", tag="wqk_sb")
        wv_sb = constp.tile([128, 2 * DV], F16, name="wv_sb", tag="wv_sb")
        b2_sb = constp.tile([128, 2], F32, name="b2_sb", tag="b2_sb")
        bv_sb = constp.tile([1, DV], F16, name="bv_sb", tag="bv_sb")
        ones_sb = constp.tile([1, 128], F16, name="ones_sb", tag="ones_sb")
        wo_sb = constp.tile([D, HID], F16, name="wo_sb", tag="wo_sb")
        scr = constp.tile([1, 1], F32, name="scr", tag="scr")

        # persistent activations (subregion deps make slices per-chunk)
        xc0 = [bigp.tile([128, CH], F16, name=f"xc0_{c}", tag=f"xc0_{c}")
               for c in range(NCH)]
        xc1 = [bigp.tile([128, CH], F16, name=f"xc1_{c}", tag=f"xc1_{c}")
               for c in range(NCH)]
        qTt = bigp.tile([128, S], F16, name="qTt", tag="qTt")  # 0:64 q/8, 64:128 bsynT
        kTt = bigp.tile([128, S], F16, name="kTt", tag="kTt")  # 0:64 k,   64:128 onehotT
        vb = bigp.tile([128, NKT * DV], F16, name="vb", tag="vb")  # v' key-major
        oall = bigp.tile([D, S], F16, name="oall", tag="oall")
        l_sb = bigp.tile([1, S], F32, name="l_sb", tag="l_sb")

        # ---- DMA issues. Rules learned from traces:
        # 1. Dependencies on DMA completions get coarsened by semaphore
        #    ring reuse, so ALL dma_starts must be emitted in global
        #    deadline order — a late-needed transfer emitted early poisons
        #    the waits of critical ones.
        # 2. The shared DMA engines are bandwidth-limited early; bulk
        #    transfers issued up-front crowd out the chunk-0 criticals.
        #    So only the critical wave is issued here; the chunk 2..7
        #    trios are issued from inside the stream (gpsimd queue, which
        #    has its own semaphore pool and is otherwise idle).
        # 3. The scalar queue only carries issues that complete before the
        #    exp stream starts (it is the ACT/bottleneck queue).
        nc.sync.dma_start(wqk_sb[:], wqk[:])
        nc.scalar.dma_start(xc1[0][:], xT[128:256, 0:CH])
        nc.sync.dma_start(xc0[0][:], xT[0:128, 0:CH])
        nc.scalar.dma_start(kTt[64:128, 0:CH], onehotT[:, 0:CH])
        nc.sync.dma_start(qTt[64:128, 0:CH], bsynT[:, 0:CH])
        nc.scalar.dma_start(b2_sb[:], bias2[:])
        nc.scalar.dma_start(wv_sb[:], wv2[:])
        nc.scalar.dma_start(bv_sb[:], bvrow[:])
        nc.scalar.dma_start(ones_sb[:], ones1[:])
        nc.sync.dma_start(xc0[1][:], xT[0:128, CH:2 * CH])
        nc.sync.dma_start(xc1[1][:], xT[128:256, CH:2 * CH])
        nc.sync.dma_start(qTt[64:128, CH:2 * CH], bsynT[:, CH:2 * CH])
        nc.sync.dma_start(kTt[64:128, CH:2 * CH], onehotT[:, CH:2 * CH])
        for c in range(2, NCH):
            cs = slice(c * CH, (c + 1) * CH)
            nc.sync.dma_start(xc0[c][:], xT[0:128, cs])
            nc.sync.dma_start(xc1[c][:], xT[128:256, cs])

        def emit_bias_dma(c):
            cs = slice(c * CH, (c + 1) * CH)
            nc.gpsimd.dma_start(qTt[64:128, cs], bsynT[:, cs])
            nc.gpsimd.dma_start(kTt[64:128, cs], onehotT[:, cs])

        # warm the ACT exp table (~2.7us) while projections run
        nc.scalar.activation(scr[:], b2_sb[0:1, 0:1], Exp)

        with (
            tc.tile_pool(name="psB", bufs=2, space="PSUM") as psB,
            tc.tile_pool(name="psAcc", bufs=2, space="PSUM") as psAcc,
            tc.tile_pool(name="psX", bufs=2, space="PSUM") as psX,
        ):
            # ---- helpers ----
            def emit_qk(c, pool, on_act=False):
                cs = slice(c * CH, (c + 1) * CH)
                qkp = pool.tile([128, CH], F32, name="qkp",
                                tag="oacc" if pool is psAcc else "px")
                mm(qkp[:], wqk_sb[:, 0:128], xc0[c][:], start=True, stop=False)
                mm(qkp[:], wqk_sb[:, 128:256], xc1[c][:], start=False,
                   stop=True)
                # kT eviction FIRST: during qb0 the next score group waits
                # only on kTt (qTt chunk c isn't read until qb c), so the
                # qT eviction stays off the critical chain.
                nc.vector.tensor_scalar_add(kTt[0:D, cs], qkp[D:128, :],
                                            b2_sb[D:128, 0:1])
                nc.vector.tensor_scalar_add(qTt[0:D, cs], qkp[0:D, :],
                                            b2_sb[0:D, 0:1])

            def emit_v(c):
                # v' computed DIRECTLY key-major: out[key, d] with the x
                # chunk slice as stationary and the wv half as moving — only
                # 3x68 moving rows per key tile (vs 512-row projections plus
                # PE transposes). The [bv | 1] row rides in as a rank-1
                # matmul against a ones row, which also plants the
                # denominator ones column.
                vtr = psX.tile([128, 4 * DV], F32, name="vtr", tag="px")
                for m in range(4):
                    ks = slice(m * KT, (m + 1) * KT)
                    vs = slice(m * DV, (m + 1) * DV)
                    mm(vtr[:, vs], xc0[c][:, ks], wv_sb[:, 0:DV],
                       start=True, stop=False)
                    mm(vtr[:, vs], xc1[c][:, ks], wv_sb[:, DV:2 * DV],
                       start=False, stop=False)
                    mm(vtr[:, vs], ones_sb[:], bv_sb[:],
                       start=False, stop=True)
                nc.vector.tensor_copy(vb[:, 4 * c * DV:(4 * c + 4) * DV],
                                      vtr[:])

            oaccs = {}

            def proj_steps(qb):
                """Deferred output projection for query block qb; the oacc
                eviction happens immediately (DVE is idle), the PE matmuls
                run later in loose slots of the next block."""
                qsl = slice(qb * QB, (qb + 1) * QB)
                oacc = oaccs.pop(qb)
                nc.vector.tensor_copy(oall[:, qsl], oacc[0:D, :])
                nc.vector.tensor_copy(l_sb[:, qsl], oacc[LCOL:LCOL + 1, :])

                def s1():
                    pj = psX.tile([128, QB], F32, name="pj", tag="px")
                    ob = obp.tile([128, QB], F16, name="ob", tag="ob")
                    mm(pj[:], wo_sb[:, 0:128], oall[:, qsl],
                       start=True, stop=True)
                    nc.vector.tensor_copy(ob[:], pj[:])
                    nc.gpsimd.dma_start(outT[0:128, qsl], ob[:])

                def s2():
                    pj = psX.tile([128, QB], F32, name="pj", tag="px")
                    ob = obp.tile([128, QB], F16, name="ob", tag="ob")
                    mm(pj[:], wo_sb[:, 128:256], oall[:, qsl],
                       start=True, stop=True)
                    nc.vector.tensor_copy(ob[:], pj[:])
                    nc.gpsimd.dma_start(outT[128:256, qsl], ob[:])
                    nc.gpsimd.dma_start(lT[:, qsl], l_sb[:, qsl])

                return [s1, s2]

            def proj_last(qb):
                """Final block: same halves; casts split across Vector and
                GpSimd so they run in parallel, and the output DMAs go on
                the two hwdge queues (sync + the now-idle scalar) so no
                slow swdge drain sits at the very end."""
                qsl = slice(qb * QB, (qb + 1) * QB)
                oacc = oaccs.pop(qb)
                nc.vector.tensor_copy(oall[:, qsl], oacc[0:D, :])
                nc.vector.tensor_copy(l_sb[:, qsl], oacc[LCOL:LCOL + 1, :])
                nc.sync.dma_start(lT[:, qsl], l_sb[:, qsl])
                for half, ofs in ((0, 0), (1, 128)):
                    pj = psX.tile([128, QB], F32, name="pjl", tag="px")
                    ob = obp.tile([128, QB], F16, name="obl", tag="ob")
                    mm(pj[:], wo_sb[:, ofs:ofs + 128], oall[:, qsl],
                       start=True, stop=True)
                    if half == 0:
                        nc.vector.tensor_copy(ob[:], pj[:])
                    else:
                        # ACT is idle once the exp stream ends — use it for
                        # the second cast so the two halves run in parallel.
                        nc.scalar.activation(
                            ob[:], pj[:], mybir.ActivationFunctionType.Copy)
                    q0 = qb * QB
                    eng = (nc.sync, nc.scalar)
                    eng[half].dma_start(outT[ofs:ofs + 128, q0:q0 + 256],
                                        ob[:, 0:256])
                    eng[1 - half].dma_start(
                        outT[ofs:ofs + 128, q0 + 256:q0 + 512],
                        ob[:, 256:512])

            # PE p-state warmup: dummy matmuls on the first-arrived weights
            # bridge the gap until the x chunk-0 DMA lands (so qk0 doesn't
            # run at the cold 0.65GHz p-state). A few more after qk0 keep
            # the PE busy while the DVE bias-add produces qTt/kTt chunk 0,
            # preserving the p-state ramp into the score stream.
            warm = psX.tile([128, 256], F32, name="warm", tag="px")
            for _ in range(10):
                mm(warm[:], wqk_sb[:, 0:128], wqk_sb[:], start=True, stop=True)
            emit_qk(0, psX)
            for _ in range(4):
                mm(warm[:], wqk_sb[:, 0:128], wqk_sb[:], start=True, stop=True)

            # ---- injected work, placed just ahead of each deadline:
            # kTt chunk c feeds score groups 2c..2c+1 -> qk(c) at group
            # 2c-1; vb chunk c is first read by PV(2c) which drains at
            # group 2c+PVLAG -> emit_v(c) at 2c+2. PV is lagged by a deep
            # PVLAG=10 so qb0 carries no PV work at all -> the PE (which
            # also runs all the injected projections at the not-yet-ramped
            # p-state) can keep the score stream ahead of ACT.
            inject = {
                0: [lambda: emit_bias_dma(2)],
                1: [lambda: emit_bias_dma(3), lambda: emit_qk(1, psAcc)],
                2: [lambda: emit_bias_dma(4), lambda: emit_v(0)],
                3: [lambda: emit_qk(2, psX)],
                4: [lambda: emit_bias_dma(5), lambda: emit_v(1)],
                5: [lambda: emit_qk(3, psAcc)],
                6: [lambda: emit_bias_dma(6), lambda: emit_v(2)],
                7: [lambda: emit_qk(4, psAcc)],
                8: [lambda: emit_bias_dma(7), lambda: emit_v(3)],
                9: [lambda: emit_qk(5, psAcc)],
                10: [lambda: nc.gpsimd.dma_start(wo_sb[:], woT[:]),
                     lambda: emit_v(4)],
                11: [lambda: emit_qk(6, psX)],
                12: [lambda: emit_v(5)],
                13: [lambda: emit_qk(7, psX)],
                14: [lambda: emit_v(6)],
            }
            inject_qb1 = {
                0: [lambda: emit_v(7)],
            }

            # ---- attention stream (PV lags scores by PVLAG groups) ----
            PVLAG = 10
            pv_queue = []
            pending_proj = []

            def emit_pv(qb, gi, p3):
                if gi == 0:
                    oaccs[qb] = psAcc.tile([DV, QB], F32, name="oacc",
                                           tag="oacc")
                oacc = oaccs[qb]
                for m in range(GRP):
                    j = GRP * gi + m
                    mm(oacc[:], vb[:, j * DV:(j + 1) * DV],
                       p3[:, m * QB:(m + 1) * QB],
                       start=(j == 0), stop=(j == NKT - 1))

            done_qb = {}

            def drain_one_pv():
                qb0_, gi0_, p30_ = pv_queue.pop(0)
                emit_pv(qb0_, gi0_, p30_)
                if gi0_ == NG - 1:
                    done_qb[qb0_] = True

            for qb in range(NQB):
                qsl = slice(qb * QB, (qb + 1) * QB)
                for gi in range(NG):
                    if qb == 0:
                        for thunk in inject.get(gi, ()):
                            thunk()
                    elif qb == 1:
                        for thunk in inject_qb1.get(gi, ()):
                            thunk()
                    s3 = psB.tile([128, GRP * QB], F32, name="s3", tag="s3")
                    for m in range(GRP):
                        j = GRP * gi + m
                        jl = slice(j * KT, (j + 1) * KT)
                        mm(s3[:, m * QB:(m + 1) * QB], kTt[:, jl], qTt[:, qsl],
                           start=True, stop=True)
                    p3 = ptp.tile([128, GRP * QB], F16, name="p3", tag="p3")
                    nc.scalar.activation(p3[:], s3[:], Exp)
                    pv_queue.append((qb, gi, p3))
                    # Last three blocks: taper the PV lag gradually (the
                    # extra drains sit mid-block, away from the boundary
                    # where the PE p-state is still recovering) so the
                    # post-stream tail is one PV group instead of PVLAG+1.
                    if qb < NQB - 3:
                        lag = PVLAG
                    elif qb == NQB - 3:
                        lag = PVLAG - min(2, max(0, gi - 8))
                    elif qb == NQB - 2:
                        lag = 8 - min(3, max(0, gi - 5))
                    else:
                        lag = 5 - min(4, max(0, gi - 4))
                    drains = 0
                    while len(pv_queue) > lag and drains < 2:
                        drain_one_pv()
                        drains += 1
                    # previous block done accumulating? evict + start proj
                    # (extend, never replace: with the taper, done_qb can
                    # fire while the previous block's s2 is still pending)
                    if done_qb.pop(qb - 1, None):
                        pending_proj.extend(proj_steps(qb - 1))
                    if pending_proj and (gi % 6 == 5 or
                                         (qb == NQB - 1 and gi % 3 == 2)):
                        pending_proj.pop(0)()
            while pv_queue:
                drain_one_pv()
            for step in pending_proj:
                step()
            proj_last(NQB - 1)


_NC_CACHE = {}


def _get_program():
    if "nc" not in _NC_CACHE:
        _NC_CACHE["nc"] = build_program()
    return _NC_CACHE["nc"]


def make_in_maps(x, codons, syn_bias, wq, bq, wk, bk, wv, bv, wo):
    in_maps = []
    for core in range(8):
        b, h = divmod(core, NH)
        hsl = slice(h * D, (h + 1) * D)
        cod = codons[b]
        onehotT = np.zeros((D, S), np.float16)
        onehotT[cod, np.arange(S)] = 1.0
        # [wqT/8 | wkT] packed as [hi-half ; lo-half] -> [128, 256]
        wqk_full = np.concatenate([wq[hsl, :].T / 8.0, wk[hsl, :].T], axis=1)
        wqk = np.concatenate([wqk_full[0:128], wqk_full[128:256]], axis=1)
        wvp = np.concatenate(
            [wv[hsl, :].T, np.zeros((HID, DV - D), np.float32)], axis=1)
        wv2 = np.concatenate([wvp[0:128], wvp[128:256]], axis=1)
        bias2 = np.zeros((128, 2), np.float32)
        bias2[:, 0] = np.concatenate([bq[hsl] / 8.0, bk[hsl]])
        bvrow = np.zeros((1, DV), np.float32)
        bvrow[0, 0:D] = bv[hsl]
        bvrow[0, LCOL] = 1.0
        in_maps.append({
            "xT": x[b].T.astype(np.float16),
            "wqk": wqk.astype(np.float16),
            "wv2": wv2.astype(np.float16),
            "bias2": bias2,
            "bvrow": bvrow.astype(np.float16),
            "ones1": np.ones((1, 128), np.float16),
            "bsynT": np.ascontiguousarray(syn_bias.T[:, cod]).astype(np.float16),
            "onehotT": onehotT,
            "woT": wo[:, hsl].T.astype(np.float16),
        })
    return in_maps


def kernel_run(inputs, trace=False):
    x = np.asarray(inputs["x"], np.float32)
    codons = np.asarray(inputs["codons"]).astype(np.int64)
    syn_bias = np.asarray(inputs["syn_bias"], np.float32)
    wq = np.asarray(inputs["wq"], np.float32)
    bq = np.asarray(inputs["bq"], np.float32)
    wk = np.asarray(inputs["wk"], np.float32)
    bk = np.asarray(inputs["bk"], np.float32)
    wv = np.asarray(inputs["wv"], np.float32)
    bv = np.asarray(inputs["bv"], np.float32)
    wo = np.asarray(inputs["wo"], np.float32)
    bo = np.asarray(inputs["bo"], np.float32)

    nc = _get_program()
    in_maps = make_in_maps(x, codons, syn_bias, wq, bq, wk, bk, wv, bv, wo)
    res = run_bass_kernel_spmd(nc, in_maps, core_ids=list(range(8)), trace=trace)

    out = np.empty((B, S, HID), np.float32)
    for b in range(B):
        acc = None
        for h in range(NH):
            r = res.results[NH * b + h]
            part = r["outT"].astype(np.float32) / r["lT"]   # normalize per head
            acc = part if acc is None else acc + part
        out[b] = acc.T + bo
    return out, res


def kernel(**inputs):
    out, _ = kernel_run(inputs, trace=False)
    return out

